# revision 1
# baseline (speedup 1.0000x reference)
"""Trainium2 Bass kernel for nn_GSA_74045236183284 (histogram_binning).

Sharding: data-parallel over batch B=8 across 8 NeuronCores (1 sample/core).
All params replicated. Zero collectives: the BatchNorm batch-statistics
coupling reduces to var_b[c] = mean_b(v/(v+eps)) with v = per-sample instance
variance; using the local sample's value deviates by <3e-6 relative (verified),
far below tolerance. InstanceNorm statistics are computed in closed form from
bin sums/counts and sum(x)/sum(x^2), avoiding extra passes over the 8MB stream.

Per-core pipeline:
  load x [128,16384] -> masks from tanh(logits) (two layouts) -> per-bin sums
  via PE transposes + accumulating matmuls -> 3 tiny attention blocks ->
  closed-form instance/batch-norm affine -> scatter-reconstruct + gelu-sum pass
  -> SE gates (fc1/fc2) -> gelu+conv+scale output pass.
"""

import sys

for _p in ("/opt/trn_rl_repo",):
    if _p not in sys.path:
        sys.path.insert(0, _p)

import numpy as np

import concourse.bass as bass
import concourse.bacc as bacc
import concourse.mybir as mybir
import concourse.tile as tile
from concourse.bass_utils import run_bass_kernel_spmd

F32 = mybir.dt.float32
AF = mybir.ActivationFunctionType
ALU = mybir.AluOpType
AX = mybir.AxisListType

B, C, N, K = 8, 128, 16384, 8
NCORES = 8
LOADCH = 1024   # x load chunk (16 chunks)
CH = 512        # scatter/conv chunk (32 chunks)


def build_nc():
    nc = bacc.Bacc("TRN2", target_bir_lowering=False, debug=False,
                   num_devices=NCORES)

    x_d = nc.dram_tensor("x", [C, N], F32, kind="ExternalInput")
    logits_d = nc.dram_tensor("logits", [N], F32, kind="ExternalInput")
    ident_d = nc.dram_tensor("ident", [C, C], F32, kind="ExternalInput")
    w_d = {}
    for nm in ("Wq1", "Wk1", "Wv1", "Wq2", "Wk2", "Wv2", "Wq3", "Wk3", "Wv3",
               "conv0_w"):
        w_d[nm] = nc.dram_tensor(nm, [C, C], F32, kind="ExternalInput")
    fc1w_d = nc.dram_tensor("fc1_w", [C // 2, C], F32, kind="ExternalInput")
    fc2w_d = nc.dram_tensor("fc2_w", [C, C // 2], F32, kind="ExternalInput")
    vecs = {}
    for nm in ("ln_w", "ln_b", "bn_w", "bn_b", "conv0_b", "fc2_b"):
        vecs[nm] = nc.dram_tensor(nm, [C], F32, kind="ExternalInput")
    vecs["fc1_b"] = nc.dram_tensor("fc1_b", [C // 2], F32, kind="ExternalInput")
    out_d = nc.dram_tensor("out", [C, N], F32, kind="ExternalOutput")

    with tile.TileContext(nc) as tc:
        _body(tc, nc, x_d, logits_d, ident_d, w_d, fc1w_d, fc2w_d, vecs, out_d)

    nc.compile()
    return nc


def _body(tc, nc, x_d, logits_d, ident_d, w_d, fc1w_d, fc2w_d, vecs, out_d):
    from contextlib import ExitStack
    ctx = ExitStack()
    with ctx:
        singles = ctx.enter_context(tc.tile_pool(name="singles", bufs=1))
        xpool = ctx.enter_context(tc.tile_pool(name="xpool", bufs=1))
        sc2 = ctx.enter_context(tc.tile_pool(name="sc2", bufs=2))
        xtp = ctx.enter_context(tc.tile_pool(name="xtp", bufs=4))
        gch = ctx.enter_context(tc.tile_pool(name="gch", bufs=3))
        och = ctx.enter_context(tc.tile_pool(name="och", bufs=3))
        dramp = ctx.enter_context(tc.tile_pool(name="dramp", bufs=1, space="DRAM"))
        psA = ctx.enter_context(tc.tile_pool(name="psA", bufs=3, space="PSUM"))
        psB = ctx.enter_context(tc.tile_pool(name="psB", bufs=1, space="PSUM"))
        psC = ctx.enter_context(tc.tile_pool(name="psC", bufs=2, space="PSUM"))
        psD = ctx.enter_context(tc.tile_pool(name="psD", bufs=2, space="PSUM"))

        # ---------------- constants / small loads ----------------
        ident = singles.tile([C, C], F32)
        nc.sync.dma_start(ident[:], ident_d.ap())
        ones_col = singles.tile([C, 1], F32)
        nc.vector.memset(ones_col[:], 1.0)
        ones_row = singles.tile([1, C], F32)
        nc.vector.memset(ones_row[:], 1.0)
        ones8 = singles.tile([K, 1], F32)
        nc.vector.memset(ones8[:], 1.0)

        lg = singles.tile([C, C], F32)   # logits as [p, f], n = p*128+f
        nc.gpsimd.dma_start(lg[:], logits_d.ap().rearrange("(p f) -> p f", f=C))

        # x: 16 chunk tiles of [128, 1024]
        xt = []
        for ci in range(N // LOADCH):
            t = xpool.tile([C, LOADCH], F32, tag=f"x{ci}")
            nc.sync.dma_start(t[:], x_d.ap()[:, ci * LOADCH:(ci + 1) * LOADCH])
            xt.append(t)

        # weight loads + transposes (WqT|WkT packed per layer)
        wsb = {}
        for nm in w_d:
            t = singles.tile([C, C], F32, tag=f"wl_{nm}")
            nc.sync.dma_start(t[:], w_d[nm].ap())
            wsb[nm] = t
        fc1w = singles.tile([C // 2, C], F32)
        nc.sync.dma_start(fc1w[:], fc1w_d.ap())
        fc2w = singles.tile([C, C // 2], F32)
        nc.sync.dma_start(fc2w[:], fc2w_d.ap())

        wqkT = []
        wvT = []
        for l in range(3):
            qk = singles.tile([C, 2 * C], F32, tag=f"wqkT{l}")
            for s, nm in enumerate((f"Wq{l+1}", f"Wk{l+1}")):
                ps = psA.tile([C, C], F32, tag="pa")
                nc.tensor.transpose(ps[:], wsb[nm][:], ident[:])
                nc.scalar.copy(qk[:, s * C:(s + 1) * C], ps[:])
            wqkT.append(qk)
            vt = singles.tile([C, C], F32, tag=f"wvT{l}")
            ps = psA.tile([C, C], F32, tag="pa")
            nc.tensor.transpose(ps[:], wsb[f"Wv{l+1}"][:], ident[:])
            nc.scalar.copy(vt[:], ps[:])
            wvT.append(vt)
        convwT = singles.tile([C, C], F32)
        ps = psA.tile([C, C], F32, tag="pa")
        nc.tensor.transpose(ps[:], wsb["conv0_w"][:], ident[:])
        nc.scalar.copy(convwT[:], ps[:])
        fc1wT = singles.tile([C, C // 2], F32)
        ps = psA.tile([C, C], F32, tag="pa")
        nc.tensor.transpose(ps[:, :C // 2], fc1w[:], ident[:C // 2, :C // 2])
        nc.scalar.copy(fc1wT[:], ps[:, :C // 2])
        fc2wT = singles.tile([C // 2, C], F32)
        ps = psA.tile([C, C], F32, tag="pa")
        nc.tensor.transpose(ps[:C // 2, :], fc2w[:], ident[:])
        nc.scalar.copy(fc2wT[:], ps[:C // 2, :])

        # vectors: bn_w/bn_b as rows; ln_w/ln_b/conv0_b/fc2_b/fc1_b -> cols
        bnw_row = singles.tile([1, C], F32)
        nc.gpsimd.dma_start(bnw_row[:], vecs["bn_w"].ap()[None, :])
        bnb_row = singles.tile([1, C], F32)
        nc.gpsimd.dma_start(bnb_row[:], vecs["bn_b"].ap()[None, :])
        vrows = singles.tile([5, C], F32)
        nc.vector.memset(vrows[:], 0.0)
        for r, nm in enumerate(("ln_w", "ln_b", "conv0_b", "fc2_b")):
            nc.gpsimd.dma_start(vrows[r:r + 1, :],
                                vecs[nm].ap()[None, :])
        nc.gpsimd.dma_start(vrows[4:5, :C // 2],
                            vecs["fc1_b"].ap()[None, :])
        ps = psA.tile([C, C], F32, tag="pa")
        nc.tensor.transpose(ps[:, :5], vrows[:], ident[:5, :5])
        vcols = singles.tile([C, 5], F32)
        nc.scalar.copy(vcols[:], ps[:, :5])
        lnw_c, lnb_c = vcols[:, 0:1], vcols[:, 1:2]
        convb_c, fc2b_c = vcols[:, 2:3], vcols[:, 3:4]
        fc1b_c = vcols[:C // 2, 4:5]

        # ---------------- masks ----------------
        # w = tanh(logits); bins (l, l+0.25], l = -1 + 0.25*j; bin 3 needs w!=0
        wA = singles.tile([C, C], F32)
        nc.scalar.activation(wA[:], lg[:], AF.Tanh)

        def build_masks(dst, src, nbins):
            # dst[:, j*128:(j+1)*128] = mask_j computed from src [128,128]
            for j in range(8):
                lo = -1.0 + 0.25 * j
                nc.vector.tensor_scalar(dst[:, j * C:(j + 1) * C], src[:],
                                        float(lo), None, ALU.is_gt)
            for j in range(7):
                nc.vector.tensor_tensor(dst[:, j * C:(j + 1) * C],
                                        dst[:, j * C:(j + 1) * C],
                                        dst[:, (j + 1) * C:(j + 2) * C],
                                        ALU.subtract)
            neq = sc2.tile([C, C], F32, tag="neq")
            nc.vector.tensor_scalar(neq[:], src[:], 0.0, None, ALU.not_equal)
            nc.vector.tensor_tensor(dst[:, 3 * C:4 * C], dst[:, 3 * C:4 * C],
                                    neq[:], ALU.mult)
            if nbins > 8:
                nc.vector.memset(dst[:, 8 * C:9 * C], 1.0)

        mA = singles.tile([C, 8 * C], F32)     # A-layout: [p, j*128+f]
        build_masks(mA, wA, 8)

        # Mrow via DRAM roundtrip: mrow_dram[j, n] with n = p*128+f
        mrow_dram = dramp.tile([K, N], F32)
        for j in range(K):
            nc.gpsimd.dma_start(mrow_dram[j:j + 1, :].rearrange("o n -> (o n)"),
                                mA[:, j * C:(j + 1) * C])
        Mrow = singles.tile([K, N], F32)
        nc.sync.dma_start(Mrow[:], mrow_dram[:])

        # nums: per-bin counts. numsA_part[p, j] = sum_f mA[p, j*128+f]
        numsA = singles.tile([C, K], F32)
        for j in range(K):
            nc.vector.reduce_sum(numsA[:, j:j + 1], mA[:, j * C:(j + 1) * C],
                                 axis=AX.X)
        nums_ps = psD.tile([K, 1], F32, tag="pd")
        nc.tensor.matmul(nums_ps[:], numsA[:], ones_col[:], start=True,
                         stop=True)
        nums_c = singles.tile([K, 1], F32)   # counts, col [j, 1]
        nc.vector.tensor_copy(nums_c[:], nums_ps[:])
        rnums_c = singles.tile([K, 1], F32)
        nc.vector.tensor_scalar(rnums_c[:], nums_c[:], 1.0, None, ALU.max)
        nc.vector.reciprocal(rnums_c[:], rnums_c[:])

        # T-layout masks from wT (for pooled lhsT), with ones column block
        wT_ps = psA.tile([C, C], F32, tag="pa")
        nc.tensor.transpose(wT_ps[:], wA[:], ident[:])
        wT = singles.tile([C, C], F32)
        nc.scalar.copy(wT[:], wT_ps[:])
        mT = singles.tile([C, 9 * C], F32)     # [i, j*128 + q]; j=8 -> ones
        build_masks(mT, wT, 9)

        # ---------------- x sumsq (during load) ----------------
        xsq_part = singles.tile([C, N // LOADCH], F32)
        for ci in range(N // LOADCH):
            scr = sc2.tile([C, LOADCH], F32, tag="sqscr")
            nc.scalar.activation(scr[:], xt[ci][:], AF.Square,
                                 accum_out=xsq_part[:, ci:ci + 1])

        # ---------------- pooled: transposes + accumulating matmuls --------
        # pooledT[j, c] (j=8 row = sum_x) accumulated over 128 chunks of n
        pooledT_ps = psB.tile([K + 1, C], F32)
        for q in range(C):
            ci, sub = divmod(q, LOADCH // C)
            xs = xt[ci][:, sub * C:(sub + 1) * C]
            tp = psA.tile([C, C], F32, tag="pa")
            nc.tensor.transpose(tp[:], xs, ident[:])
            xT = xtp.tile([C, C], F32, tag="xTsb")
            if q % 2 == 0:
                nc.scalar.copy(xT[:], tp[:])
            else:
                nc.vector.tensor_copy(xT[:], tp[:])
            nc.tensor.matmul(pooledT_ps[:], mT[:, q::C], xT[:],
                             start=(q == 0), stop=(q == C - 1))
        pooledT9 = singles.tile([K + 1, C], F32)
        nc.vector.tensor_copy(pooledT9[:], pooledT_ps[:])
        pooledT = pooledT9[:K, :]
        sumx_dram = dramp.tile([1, C], F32, tag="sxd")
        nc.gpsimd.dma_start(sumx_dram[:], pooledT9[K:K + 1, :])
        sumx_row = singles.tile([1, C], F32)
        nc.gpsimd.dma_start(sumx_row[:], sumx_dram[:])

        featT = singles.tile([K, C], F32)
        nc.vector.tensor_scalar(featT[:], pooledT, rnums_c[:], None,
                                ALU.mult)

        # ---------------- attention x3 (fea orientation [c, j]) -----------
        fea = singles.tile([C, K], F32, tag="fea0")
        fps = psA.tile([C, C], F32, tag="pa")
        nc.tensor.transpose(fps[:, :K], featT[:], ident[:K, :K])
        nc.vector.tensor_copy(fea[:], fps[:, :K])

        temp = float(np.sqrt(np.float32(C)))
        for l in range(3):
            qk_ps = psD.tile([K, 2 * C], F32, tag="pd")
            nc.tensor.matmul(qk_ps[:], fea[:], wqkT[l][:], start=True,
                             stop=True)
            qkT = singles.tile([K, 2 * C], F32, tag=f"qkT{l}")
            nc.scalar.activation(qkT[:, :C], qk_ps[:, :C], AF.Copy,
                                 scale=1.0 / temp)
            nc.vector.tensor_copy(qkT[:, C:], qk_ps[:, C:])
            v_ps = psD.tile([C, K], F32, tag="pd")
            nc.tensor.matmul(v_ps[:], wvT[l][:], fea[:], start=True, stop=True)
            vsb = singles.tile([C, K], F32, tag=f"v{l}")
            nc.vector.tensor_copy(vsb[:], v_ps[:])

            at_ps = psA.tile([C, C], F32, tag="pa")
            nc.tensor.matmul(at_ps[:], qkT[:, :C], qkT[:, C:], start=True,
                             stop=True)
            esb = singles.tile([C, C], F32, tag=f"e{l}")
            sume = singles.tile([C, 1], F32, tag=f"se{l}")
            nc.scalar.activation(esb[:], at_ps[:], AF.Exp, accum_out=sume[:])
            rse = singles.tile([C, 1], F32, tag=f"rse{l}")
            nc.vector.reciprocal(rse[:], sume[:])
            eT_ps = psA.tile([C, C], F32, tag="pa")
            nc.tensor.transpose(eT_ps[:], esb[:], ident[:])
            eT = singles.tile([C, C], F32, tag=f"eT{l}")
            nc.scalar.copy(eT[:], eT_ps[:])
            ao_ps = psD.tile([C, K], F32, tag="pd")
            nc.tensor.matmul(ao_ps[:], eT[:], vsb[:], start=True, stop=True)

            osb = singles.tile([C, 2 * K], F32, tag=f"osb{l}")
            nc.vector.tensor_scalar(osb[:, :K], ao_ps[:], rse[:], None,
                                    ALU.mult)
            nc.vector.tensor_tensor(osb[:, :K], osb[:, :K], fea[:], ALU.add)
            nc.scalar.activation(osb[:, K:], osb[:, :K], AF.Square)
            st_ps = psD.tile([1, 2 * K], F32, tag="pd")
            nc.tensor.matmul(st_ps[:], ones_col[:], osb[:], start=True,
                             stop=True)
            mr = singles.tile([1, 2 * K], F32, tag=f"mr{l}")
            nc.vector.tensor_scalar(mr[:], st_ps[:], 1.0 / C, None, ALU.mult)
            musq = singles.tile([1, K], F32, tag=f"musq{l}")
            nc.scalar.activation(musq[:], mr[:, :K], AF.Square)
            nc.vector.tensor_tensor(mr[:, K:], mr[:, K:], musq[:],
                                    ALU.subtract)
            nc.vector.tensor_scalar(mr[:, K:], mr[:, K:], 1e-6, None, ALU.add)
            nc.vector.reciprocal(mr[:, K:], mr[:, K:])
            nc.scalar.activation(mr[:, K:], mr[:, K:], AF.Sqrt)
            bc_ps = psD.tile([C, 2 * K], F32, tag="pd")
            nc.tensor.matmul(bc_ps[:], ones_row[:], mr[:], start=True,
                             stop=True)
            fea2 = singles.tile([C, K], F32, tag=f"fea{l+1}")
            nc.vector.tensor_tensor(fea2[:], osb[:, :K], bc_ps[:, :K],
                                    ALU.subtract)
            nc.vector.tensor_tensor(fea2[:], fea2[:], bc_ps[:, K:], ALU.mult)
            nc.vector.tensor_scalar(fea2[:], fea2[:], lnw_c, lnb_c,
                                    ALU.mult, ALU.add)
            fea = fea2

        # exit transpose: featT_f [j, c] (+ squared) for scatter & stats
        ftp = psA.tile([C, C], F32, tag="pa")
        nc.tensor.transpose(ftp[:K, :], fea[:], ident[:])
        featT2 = singles.tile([K, 2 * C], F32)   # [featT | featT^2]
        nc.vector.tensor_copy(featT2[:, :C], ftp[:K, :])
        nc.scalar.activation(featT2[:, C:], featT2[:, :C], AF.Square)

        # ---------------- closed-form instance stats ----------------
        # r1 = sum_j featT*nums ; r2 = sum_j featT^2*nums ; r3 = sum_j featT*pooled
        prod = singles.tile([K, C], F32)
        nc.vector.tensor_tensor(prod[:], featT2[:, :C], pooledT,
                                ALU.mult)
        r12_ps = psD.tile([1, 2 * C], F32, tag="pd")
        nc.tensor.matmul(r12_ps[:], nums_c[:], featT2[:], start=True,
                         stop=True)
        r3_ps = psD.tile([1, C], F32, tag="pd")
        nc.tensor.matmul(r3_ps[:], ones8[:], prod[:], start=True, stop=True)

        xsq_col = singles.tile([C, 1], F32)
        nc.vector.reduce_sum(xsq_col[:], xsq_part[:], axis=AX.X)
        xsq_ps = psA.tile([C, C], F32, tag="pa")
        nc.tensor.transpose(xsq_ps[:1, :], xsq_col[:], ident[:])
        # rows: s_recon, ss_recon -> mu, var -> s_row/b_row
        srow = singles.tile([1, C], F32)
        nc.vector.tensor_tensor(srow[:], sumx_row[:], r12_ps[:, :C],
                                ALU.add)
        ssrow = singles.tile([1, C], F32)
        nc.vector.tensor_scalar(ssrow[:], r3_ps[:], 2.0, None, ALU.mult)
        nc.vector.tensor_tensor(ssrow[:], ssrow[:], r12_ps[:, C:], ALU.add)
        nc.vector.tensor_tensor(ssrow[:], ssrow[:], xsq_ps[:1, :], ALU.add)
        mu_row = singles.tile([1, C], F32)
        nc.vector.tensor_scalar(mu_row[:], srow[:], 1.0 / N, None, ALU.mult)
        var_row = singles.tile([1, C], F32)
        musq_row = singles.tile([1, C], F32)
        nc.scalar.activation(musq_row[:], mu_row[:], AF.Square)
        nc.vector.tensor_scalar(var_row[:], ssrow[:], 1.0 / N, None, ALU.mult)
        nc.vector.tensor_tensor(var_row[:], var_row[:], musq_row[:],
                                ALU.subtract)
        vpe = singles.tile([1, C], F32)
        nc.vector.tensor_scalar(vpe[:], var_row[:], 1e-5, None, ALU.add)
        inv = singles.tile([1, C], F32)
        nc.vector.reciprocal(inv[:], vpe[:])
        rs_i = singles.tile([1, C], F32)
        nc.scalar.activation(rs_i[:], inv[:], AF.Sqrt)
        rbn = singles.tile([1, C], F32)          # var/(var+eps)
        nc.vector.tensor_tensor(rbn[:], var_row[:], inv[:], ALU.mult)
        nc.vector.tensor_scalar(rbn[:], rbn[:], 1e-5, None, ALU.add)
        nc.vector.reciprocal(rbn[:], rbn[:])
        nc.scalar.activation(rbn[:], rbn[:], AF.Sqrt)  # rs_b
        s_rowt = singles.tile([1, C], F32)
        b_rowt = singles.tile([1, C], F32)
        nc.vector.tensor_tensor(s_rowt[:], rs_i[:], rbn[:], ALU.mult)
        nc.vector.tensor_tensor(s_rowt[:], s_rowt[:], bnw_row[:], ALU.mult)
        nc.vector.tensor_tensor(b_rowt[:], mu_row[:], s_rowt[:], ALU.mult)
        nc.vector.tensor_tensor(b_rowt[:], bnb_row[:], b_rowt[:],
                                ALU.subtract)
        s_ps = psD.tile([C, 1], F32, tag="pd")
        nc.tensor.transpose(s_ps[:], s_rowt[:], ident[:1, :1])
        b_ps = psD.tile([C, 1], F32, tag="pd")
        nc.tensor.transpose(b_ps[:], b_rowt[:], ident[:1, :1])
        sb_col = singles.tile([C, 2], F32)
        nc.vector.tensor_copy(sb_col[:, 0:1], s_ps[:])
        nc.vector.tensor_copy(sb_col[:, 1:2], b_ps[:])
        s_col, b_col = sb_col[:, 0:1], sb_col[:, 1:2]

        # ---------------- pass R: scatter + recon + gelu-sum ----------------
        gsum_part = singles.tile([C, N // CH], F32)
        for r in range(N // CH):
            off = r * CH
            ci, sub = divmod(off, LOADCH)
            xs = xt[ci][:, sub:sub + CH]
            sc_ps = psC.tile([C, CH], F32, tag="pc")
            nc.tensor.matmul(sc_ps[:], featT2[:, :C], Mrow[:, off:off + CH],
                             start=True, stop=True)
            nc.vector.tensor_tensor(xs, sc_ps[:], xs, ALU.add)
            g = gch.tile([C, CH], F32, tag="g")
            nc.scalar.activation(g[:], xs, AF.Gelu, bias=b_col, scale=s_col,
                                 accum_out=gsum_part[:, r:r + 1])

        # ---------------- SE gates ----------------
        gsum_col = singles.tile([C, 1], F32)
        nc.vector.reduce_sum(gsum_col[:], gsum_part[:], axis=AX.X)
        sq_ps = psD.tile([C, 1], F32, tag="pd")
        nc.tensor.matmul(sq_ps[:], convwT[:], gsum_col[:], start=True,
                         stop=True)
        sq = singles.tile([C, 1], F32)
        nc.vector.tensor_scalar(sq[:], sq_ps[:], 1.0 / N, convb_c,
                                ALU.mult, ALU.add)
        f1_ps = psD.tile([C // 2, 1], F32, tag="pd")
        nc.tensor.matmul(f1_ps[:], fc1wT[:], sq[:], start=True, stop=True)
        f1 = singles.tile([C // 2, 1], F32)
        nc.scalar.activation(f1[:], f1_ps[:], AF.Gelu, bias=fc1b_c)
        f2_ps = psD.tile([C, 1], F32, tag="pd")
        nc.tensor.matmul(f2_ps[:], fc2wT[:], f1[:], start=True, stop=True)
        f2 = singles.tile([C, 1], F32)
        nc.scalar.activation(f2[:], f2_ps[:], AF.Sigmoid, bias=fc2b_c)
        fb = singles.tile([C, 1], F32)     # f2 * conv0_b
        nc.vector.tensor_tensor(fb[:], f2[:], convb_c, ALU.mult)

        # ---------------- pass F: gelu + conv + gate + store ----------------
        for r in range(N // CH):
            off = r * CH
            ci, sub = divmod(off, LOADCH)
            xs = xt[ci][:, sub:sub + CH]
            g = gch.tile([C, CH], F32, tag="g2")
            nc.scalar.activation(g[:], xs, AF.Gelu, bias=b_col, scale=s_col)
            cv_ps = psC.tile([C, CH], F32, tag="pc")
            nc.tensor.matmul(cv_ps[:], convwT[:], g[:], start=True, stop=True)
            ot = och.tile([C, CH], F32, tag="ot")
            nc.vector.tensor_scalar(ot[:], cv_ps[:], f2[:], fb[:],
                                    ALU.mult, ALU.add)
            nc.sync.dma_start(out_d.ap()[:, off:off + CH], ot[:])


_NC_CACHE = {}


def _get_nc():
    if "nc" not in _NC_CACHE:
        _NC_CACHE["nc"] = build_nc()
    return _NC_CACHE["nc"]


def kernel(**inputs):
    x = np.ascontiguousarray(np.asarray(inputs["x"], dtype=np.float32))
    logits = np.ascontiguousarray(np.asarray(inputs["logits"],
                                             dtype=np.float32))
    assert x.shape == (B, C, N, 1) and logits.shape == (B, N)
    ident = np.eye(C, dtype=np.float32)
    shared = {"ident": ident}
    for nm in ("Wq1", "Wk1", "Wv1", "Wq2", "Wk2", "Wv2", "Wq3", "Wk3", "Wv3",
               "conv0_w", "fc1_w", "fc2_w", "ln_w", "ln_b", "bn_w", "bn_b",
               "conv0_b", "fc1_b", "fc2_b"):
        shared[nm] = np.ascontiguousarray(np.asarray(inputs[nm],
                                                     dtype=np.float32))
    in_maps = []
    for i in range(NCORES):
        m = dict(shared)
        m["x"] = np.ascontiguousarray(x[i, :, :, 0])
        m["logits"] = np.ascontiguousarray(logits[i])
        in_maps.append(m)

    nc = _get_nc()
    res = run_bass_kernel_spmd(nc, in_maps, list(range(NCORES))).results
    out = np.stack([res[i]["out"] for i in range(NCORES)], axis=0)
    return out[..., None].astype(np.float32)



# revision 7
# speedup vs baseline: 1.4469x; 1.4469x over previous
"""Trainium2 Bass kernel for nn_GSA_74045236183284 (histogram_binning).

Sharding: data-parallel over batch B=8 across 8 NeuronCores (1 sample/core).
All params replicated. Zero collectives: BatchNorm batch-variance is
approximated by the local sample's var/(var+eps) (deviation <3e-6 rel).
InstanceNorm statistics are computed in closed form from bin sums/counts and
sum(x)/sum(x^2), avoiding extra passes over the 8MB stream.

v2 optimizations vs v1 (289.7us):
 - x loaded via SWDGE casting DMA straight to bf16 (no f32 copy in SBUF);
   all streaming matmuls (transpose, pool, scatter, conv) run bf16 with FWL.
 - chunk transposes are regular matmuls against a bf16 identity (keeps PE
   HAM-warm, ~4x faster than fp32 transpose-mode path).
 - the recon add (x + scatter) is folded into the scatter matmul as an
   accumulating identity matmul -> PSUM, freeing the DVE in pass R.
 - activation table thrash fixed: 9 table loads -> 2. Front half uses only
   natural_log_exp_and_others (tanh built from exp; rsqrt = exp(-0.5*ln)),
   back half uses only gelu_and_others (sigmoid = 0.5+0.5*tanh(x/2)).
"""

import sys

for _p in ("/opt/trn_rl_repo",):
    if _p not in sys.path:
        sys.path.insert(0, _p)

import numpy as np

import concourse.bass as bass
import concourse.bacc as bacc
import concourse.mybir as mybir
import concourse.tile as tile
from concourse.bass_utils import run_bass_kernel_spmd

F32 = mybir.dt.float32
BF16 = mybir.dt.bfloat16
AF = mybir.ActivationFunctionType
ALU = mybir.AluOpType
AX = mybir.AxisListType

B, C, N, K = 8, 128, 16384, 8
NCORES = 8
LOADCH = 2048   # x cast-load chunk (8 chunks)
CH = 512        # scatter/conv chunk (32 chunks)


def build_nc():
    nc = bacc.Bacc("TRN2", target_bir_lowering=False, debug=False,
                   num_devices=NCORES)

    x_d = nc.dram_tensor("x", [C, N], F32, kind="ExternalInput")
    logits_d = nc.dram_tensor("logits", [N], F32, kind="ExternalInput")
    ident_d = nc.dram_tensor("ident", [C, C], F32, kind="ExternalInput")
    w_d = {}
    for nm in ("Wq1", "Wk1", "Wv1", "Wq2", "Wk2", "Wv2", "Wq3", "Wk3", "Wv3",
               "conv0_w"):
        w_d[nm] = nc.dram_tensor(nm, [C, C], F32, kind="ExternalInput")
    fc1w_d = nc.dram_tensor("fc1_w", [C // 2, C], F32, kind="ExternalInput")
    fc2w_d = nc.dram_tensor("fc2_w", [C, C // 2], F32, kind="ExternalInput")
    vecs = {}
    for nm in ("ln_w", "ln_b", "bn_w", "bn_b", "conv0_b", "fc2_b"):
        vecs[nm] = nc.dram_tensor(nm, [C], F32, kind="ExternalInput")
    vecs["fc1_b"] = nc.dram_tensor("fc1_b", [C // 2], F32, kind="ExternalInput")
    out_d = nc.dram_tensor("out", [C, N], F32, kind="ExternalOutput")

    with tile.TileContext(nc) as tc:
        _body(tc, nc, x_d, logits_d, ident_d, w_d, fc1w_d, fc2w_d, vecs, out_d)

    nc.compile()
    return nc


def _body(tc, nc, x_d, logits_d, ident_d, w_d, fc1w_d, fc2w_d, vecs, out_d):
    from contextlib import ExitStack
    ctx = ExitStack()
    with ctx:
        singles = ctx.enter_context(tc.tile_pool(name="singles", bufs=1))
        xpool = ctx.enter_context(tc.tile_pool(name="xpool", bufs=1))
        gpool = ctx.enter_context(tc.tile_pool(name="gpool", bufs=1))
        xtp = ctx.enter_context(tc.tile_pool(name="xtp", bufs=3))
        och = ctx.enter_context(tc.tile_pool(name="och", bufs=3))
        dramp = ctx.enter_context(tc.tile_pool(name="dramp", bufs=1, space="DRAM"))
        psT = ctx.enter_context(tc.tile_pool(name="psT", bufs=2, space="PSUM"))
        psS = ctx.enter_context(tc.tile_pool(name="psS", bufs=1, space="PSUM"))
        psB = ctx.enter_context(tc.tile_pool(name="psB", bufs=1, space="PSUM"))
        psD = ctx.enter_context(tc.tile_pool(name="psD", bufs=1, space="PSUM"))
        psR = ctx.enter_context(tc.tile_pool(name="psR", bufs=2, space="PSUM"))

        # ---------------- constants / small loads ----------------
        ident = singles.tile([C, C], F32)
        nc.sync.dma_start(ident[:], ident_d.ap())
        ident_bf = singles.tile([C, C], BF16)
        nc.vector.tensor_copy(ident_bf[:], ident[:])
        ones_col = singles.tile([C, 1], F32)
        nc.vector.memset(ones_col[:], 1.0)
        ones_row = singles.tile([1, C], F32)
        nc.vector.memset(ones_row[:], 1.0)
        ones8 = singles.tile([K, 1], F32)
        nc.vector.memset(ones8[:], 1.0)
        ones9 = singles.tile([K + 1, 1], F32)
        nc.vector.memset(ones9[:], 1.0)
        neg8 = singles.tile([K, 1], F32)
        nc.vector.memset(neg8[:], -1.0)

        lg = singles.tile([C, C], F32)   # logits as [p, f], n = p*128+f
        nc.gpsimd.dma_start(lg[:], logits_d.ap().rearrange("(p f) -> p f", f=C))

        # x: 8 chunk tiles of [128, 2048] bf16 via casting SWDGE DMA
        xt = []
        for ci in range(N // LOADCH):
            t = xpool.tile([C, LOADCH], BF16, tag=f"x{ci}")
            nc.gpsimd.dma_start(t[:], x_d.ap()[:, ci * LOADCH:(ci + 1) * LOADCH])
            xt.append(t)

        def xsl(off, width):
            ci, sub = divmod(off, LOADCH)
            return xt[ci][:, sub:sub + width]

        # weight loads + transposes (WqT|WkT packed per layer)
        wsb = {}
        for nm in w_d:
            t = singles.tile([C, C], F32, tag=f"wl_{nm}")
            nc.sync.dma_start(t[:], w_d[nm].ap())
            wsb[nm] = t
        fc1w = singles.tile([C // 2, C], F32)
        nc.sync.dma_start(fc1w[:], fc1w_d.ap())
        fc2w = singles.tile([C, C // 2], F32)
        nc.sync.dma_start(fc2w[:], fc2w_d.ap())

        wqkT = []
        wvT = []
        for l in range(3):
            qk = singles.tile([C, 2 * C], F32, tag=f"wqkT{l}")
            for s, nm in enumerate((f"Wq{l+1}", f"Wk{l+1}")):
                ps = psS.tile([C, C], F32, tag="pa")
                nc.tensor.transpose(ps[:], wsb[nm][:], ident[:])
                nc.scalar.copy(qk[:, s * C:(s + 1) * C], ps[:])
            wqkT.append(qk)
            vt = singles.tile([C, C], F32, tag=f"wvT{l}")
            ps = psS.tile([C, C], F32, tag="pa")
            nc.tensor.transpose(ps[:], wsb[f"Wv{l+1}"][:], ident[:])
            nc.scalar.copy(vt[:], ps[:])
            wvT.append(vt)
        convwT = singles.tile([C, C], F32)
        ps = psS.tile([C, C], F32, tag="pa")
        nc.tensor.transpose(ps[:], wsb["conv0_w"][:], ident[:])
        nc.scalar.copy(convwT[:], ps[:])
        convwT_bf = singles.tile([C, C], BF16)
        nc.vector.tensor_copy(convwT_bf[:], convwT[:])
        fc1wT = singles.tile([C, C // 2], F32)
        ps = psS.tile([C, C], F32, tag="pa")
        nc.tensor.transpose(ps[:, :C // 2], fc1w[:], ident[:C // 2, :C // 2])
        nc.scalar.copy(fc1wT[:], ps[:, :C // 2])
        fc2wT = singles.tile([C // 2, C], F32)
        ps = psS.tile([C, C], F32, tag="pa")
        nc.tensor.transpose(ps[:C // 2, :], fc2w[:], ident[:])
        nc.scalar.copy(fc2wT[:], ps[:C // 2, :])

        # vectors: bn_w/bn_b as rows; ln_w/ln_b/conv0_b/fc2_b/fc1_b -> cols
        bnw_row = singles.tile([1, C], F32)
        nc.gpsimd.dma_start(bnw_row[:], vecs["bn_w"].ap()[None, :])
        bnb_row = singles.tile([1, C], F32)
        nc.gpsimd.dma_start(bnb_row[:], vecs["bn_b"].ap()[None, :])
        vrows = singles.tile([5, C], F32)
        nc.vector.memset(vrows[:], 0.0)
        for r, nm in enumerate(("ln_w", "ln_b", "conv0_b", "fc2_b")):
            nc.gpsimd.dma_start(vrows[r:r + 1, :], vecs[nm].ap()[None, :])
        nc.gpsimd.dma_start(vrows[4:5, :C // 2], vecs["fc1_b"].ap()[None, :])
        ps = psS.tile([C, C], F32, tag="pa")
        nc.tensor.transpose(ps[:, :5], vrows[:], ident[:5, :5])
        vcols = singles.tile([C, 5], F32)
        nc.scalar.copy(vcols[:], ps[:, :5])
        lnw_c, lnb_c = vcols[:, 0:1], vcols[:, 1:2]
        convb_c, fc2b_c = vcols[:, 2:3], vcols[:, 3:4]
        fc1b_c = vcols[:C // 2, 4:5]
        fc2b_half = singles.tile([C, 1], F32)
        nc.vector.tensor_scalar(fc2b_half[:], fc2b_c, 0.5, None, ALU.mult)

        # ---------------- masks ----------------
        # w = tanh(logits) = 1 - 2/(exp(2*logits)+1)  (keeps ACT on the
        # natural_log_exp table set; bins (l, l+0.25], l = -1 + 0.25*j)
        wA = singles.tile([C, C], F32)
        escr = singles.tile([C, C], F32)
        nc.scalar.activation(escr[:], lg[:], AF.Exp, scale=2.0)
        nc.vector.tensor_scalar(escr[:], escr[:], 1.0, None, ALU.add)
        nc.vector.reciprocal(escr[:], escr[:])
        nc.vector.tensor_scalar(wA[:], escr[:], -2.0, 1.0, ALU.mult, ALU.add)

        def build_masks(dst, src, nbins):
            # dst[:, j*128:(j+1)*128] = mask_j computed from src [128,128]
            for j in range(8):
                lo = -1.0 + 0.25 * j
                nc.vector.tensor_scalar(dst[:, j * C:(j + 1) * C], src[:],
                                        float(lo), None, ALU.is_gt)
            for j in range(7):
                nc.vector.tensor_tensor(dst[:, j * C:(j + 1) * C],
                                        dst[:, j * C:(j + 1) * C],
                                        dst[:, (j + 1) * C:(j + 2) * C],
                                        ALU.subtract)
            neq = singles.tile([C, C], F32, tag=f"neq{nbins}")
            nc.vector.tensor_scalar(neq[:], src[:], 0.0, None, ALU.not_equal)
            nc.vector.tensor_tensor(dst[:, 3 * C:4 * C], dst[:, 3 * C:4 * C],
                                    neq[:], ALU.mult)
            if nbins > 8:
                nc.vector.memset(dst[:, 8 * C:9 * C], 1.0)

        mA = singles.tile([C, 8 * C], F32)     # A-layout: [p, j*128+f]
        build_masks(mA, wA, 8)

        # Mrow (bf16) via DRAM roundtrip with cast: mrow[j, n], n = p*128+f
        mrow_dram = dramp.tile([K, N], BF16)
        for j in range(K):
            nc.gpsimd.dma_start(mrow_dram[j:j + 1, :].rearrange("o n -> (o n)"),
                                mA[:, j * C:(j + 1) * C])
        Mrow = singles.tile([K, N], BF16)
        nc.sync.dma_start(Mrow[:], mrow_dram[:])

        # nums: per-bin counts. numsA[p, j] = sum_f mA[p, j*128+f]
        numsA = singles.tile([C, K], F32)
        for j in range(K):
            nc.vector.reduce_sum(numsA[:, j:j + 1], mA[:, j * C:(j + 1) * C],
                                 axis=AX.X)
        nums_ps = psD.tile([K, 1], F32, tag="pd1")
        nc.tensor.matmul(nums_ps[:], numsA[:], ones_col[:], start=True,
                         stop=True)
        nums_c = singles.tile([K, 1], F32)   # counts, col [j, 1]
        nc.vector.tensor_copy(nums_c[:], nums_ps[:])
        rnums_c = singles.tile([K, 1], F32)
        nc.vector.tensor_scalar(rnums_c[:], nums_c[:], 1.0, None, ALU.max)
        nc.vector.reciprocal(rnums_c[:], rnums_c[:])

        # T-layout masks from wT (pool stationary), with ones column block
        wT_ps = psS.tile([C, C], F32, tag="pa")
        nc.tensor.transpose(wT_ps[:], wA[:], ident[:])
        wT = singles.tile([C, C], F32)
        nc.scalar.copy(wT[:], wT_ps[:])
        mTf = singles.tile([C, 9 * C], F32)    # [f, j*128 + p]; j=8 -> ones
        build_masks(mTf, wT, 9)
        mT = singles.tile([C, 9 * C], BF16)
        nc.vector.tensor_copy(mT[:], mTf[:])

        # ---------------- x sumsq (bf16 stream, scalar engine) -------------
        xsq_part = singles.tile([C, N // LOADCH], F32)
        for ci in range(N // LOADCH):
            scr = xtp.tile([C, LOADCH], BF16, tag="sqscr")
            nc.scalar.activation(scr[:], xt[ci][:], AF.Square,
                                 accum_out=xsq_part[:, ci:ci + 1])

        # ---------------- pooled: bf16 transposes + accumulating matmuls ---
        # pooledT[j, c] (j=8 row = sum_x) accumulated over 128 chunks of n.
        # Transpose = regular matmul against bf16 identity (keeps PE warm).
        pooledT_ps = psB.tile([K + 1, C], F32)
        NGR = 4  # chunks per transpose group
        for g in range(C // NGR):
            tp = psT.tile([C, NGR * C], F32, tag="pt")
            for i in range(NGR):
                q = g * NGR + i
                nc.tensor.matmul(tp[:, i * C:(i + 1) * C],
                                 xsl(q * C, C), ident_bf[:],
                                 start=True, stop=True)
            xT = xtp.tile([C, NGR * C], BF16, tag="xT")
            if g % 2 == 0:
                nc.scalar.copy(xT[:], tp[:])
            else:
                nc.vector.tensor_copy(xT[:], tp[:])
            for i in range(NGR):
                q = g * NGR + i
                nc.tensor.matmul(pooledT_ps[:], mT[:, q::C],
                                 xT[:, i * C:(i + 1) * C],
                                 start=(q == 0), stop=(q == C - 1))
        pooledT9 = singles.tile([K + 1, C], F32)
        nc.vector.tensor_copy(pooledT9[:], pooledT_ps[:])
        pooledT = pooledT9[:K, :]
        # sumx_row = (sum of all 9 pooledT rows) - (sum of the 8 bin rows),
        # accumulated in one PSUM tile via a -1 stationary
        s9_ps = psD.tile([1, C], F32, tag="pd2")
        nc.tensor.matmul(s9_ps[:], ones9[:], pooledT9[:], start=True,
                         stop=False)
        nc.tensor.matmul(s9_ps[:], neg8[:], pooledT, start=False, stop=True)
        sumx_row = singles.tile([1, C], F32)
        nc.vector.tensor_copy(sumx_row[:], s9_ps[:])

        featT = singles.tile([K, C], F32)
        nc.vector.tensor_scalar(featT[:], pooledT, rnums_c[:], None,
                                ALU.mult)

        # ---------------- attention x3 (fea orientation [c, j]) -----------
        fea = singles.tile([C, K], F32, tag="fea0")
        fps = psS.tile([C, C], F32, tag="pa")
        nc.tensor.transpose(fps[:, :K], featT[:], ident[:K, :K])
        nc.vector.tensor_copy(fea[:], fps[:, :K])

        temp = float(np.sqrt(np.float32(C)))
        for l in range(3):
            qk_ps = psD.tile([K, 2 * C], F32, tag="pd1")
            nc.tensor.matmul(qk_ps[:], fea[:], wqkT[l][:], start=True,
                             stop=True)
            qkT = singles.tile([K, 2 * C], F32, tag=f"qkT{l}")
            nc.scalar.activation(qkT[:, :C], qk_ps[:, :C], AF.Copy,
                                 scale=1.0 / temp)
            nc.vector.tensor_copy(qkT[:, C:], qk_ps[:, C:])
            v_ps = psD.tile([C, K], F32, tag="pd2")
            nc.tensor.matmul(v_ps[:], wvT[l][:], fea[:], start=True, stop=True)
            vsb = singles.tile([C, K], F32, tag=f"v{l}")
            nc.vector.tensor_copy(vsb[:], v_ps[:])

            at_ps = psS.tile([C, C], F32, tag="pa")
            nc.tensor.matmul(at_ps[:], qkT[:, :C], qkT[:, C:], start=True,
                             stop=True)
            esb = singles.tile([C, C], F32, tag=f"e{l}")
            sume = singles.tile([C, 1], F32, tag=f"se{l}")
            nc.scalar.activation(esb[:], at_ps[:], AF.Exp, accum_out=sume[:])
            rse = singles.tile([C, 1], F32, tag=f"rse{l}")
            nc.vector.reciprocal(rse[:], sume[:])
            eT_ps = psS.tile([C, C], F32, tag="pa")
            nc.tensor.transpose(eT_ps[:], esb[:], ident[:])
            eT = singles.tile([C, C], F32, tag=f"eT{l}")
            nc.scalar.copy(eT[:], eT_ps[:])
            ao_ps = psD.tile([C, K], F32, tag="pd2")
            nc.tensor.matmul(ao_ps[:], eT[:], vsb[:], start=True, stop=True)

            osb = singles.tile([C, 2 * K], F32, tag=f"osb{l}")
            nc.vector.tensor_scalar(osb[:, :K], ao_ps[:], rse[:], None,
                                    ALU.mult)
            nc.vector.tensor_tensor(osb[:, :K], osb[:, :K], fea[:], ALU.add)
            nc.scalar.activation(osb[:, K:], osb[:, :K], AF.Square)
            st_ps = psD.tile([1, 2 * K], F32, tag="pd1")
            nc.tensor.matmul(st_ps[:], ones_col[:], osb[:], start=True,
                             stop=True)
            mr = singles.tile([1, 2 * K], F32, tag=f"mr{l}")
            nc.vector.tensor_scalar(mr[:], st_ps[:], 1.0 / C, None, ALU.mult)
            musq = singles.tile([1, K], F32, tag=f"musq{l}")
            nc.scalar.activation(musq[:], mr[:, :K], AF.Square)
            nc.vector.tensor_tensor(mr[:, K:], mr[:, K:], musq[:],
                                    ALU.subtract)
            nc.vector.tensor_scalar(mr[:, K:], mr[:, K:], 1e-6, None, ALU.add)
            # rstd = exp(-0.5 * ln(var+eps)) -- stays on the nl_exp table set
            nc.scalar.activation(mr[:, K:], mr[:, K:], AF.Ln)
            nc.scalar.activation(mr[:, K:], mr[:, K:], AF.Exp, scale=-0.5)
            bc_ps = psD.tile([C, 2 * K], F32, tag="pd2")
            nc.tensor.matmul(bc_ps[:], ones_row[:], mr[:], start=True,
                             stop=True)
            fea2 = singles.tile([C, K], F32, tag=f"fea{l+1}")
            nc.vector.tensor_tensor(fea2[:], osb[:, :K], bc_ps[:, :K],
                                    ALU.subtract)
            nc.vector.tensor_tensor(fea2[:], fea2[:], bc_ps[:, K:], ALU.mult)
            nc.vector.tensor_scalar(fea2[:], fea2[:], lnw_c, lnb_c,
                                    ALU.mult, ALU.add)
            fea = fea2

        # exit transpose: featT_f [j, c] (+ squared) for scatter & stats
        ftp = psS.tile([C, C], F32, tag="pa")
        nc.tensor.transpose(ftp[:K, :], fea[:], ident[:])
        featT2 = singles.tile([K, 2 * C], F32)   # [featT | featT^2]
        nc.vector.tensor_copy(featT2[:, :C], ftp[:K, :])
        nc.scalar.activation(featT2[:, C:], featT2[:, :C], AF.Square)
        featT2_bf = singles.tile([K, C], BF16)
        nc.vector.tensor_copy(featT2_bf[:], featT2[:, :C])

        # ---------------- closed-form instance stats ----------------
        # r1 = sum_j featT*nums ; r2 = sum_j featT^2*nums ; r3 = sum_j featT*pooled
        prod = singles.tile([K, C], F32)
        nc.vector.tensor_tensor(prod[:], featT2[:, :C], pooledT,
                                ALU.mult)
        r12_ps = psD.tile([1, 2 * C], F32, tag="pd1")
        nc.tensor.matmul(r12_ps[:], nums_c[:], featT2[:], start=True,
                         stop=True)
        r3_ps = psD.tile([1, C], F32, tag="pd2")
        nc.tensor.matmul(r3_ps[:], ones8[:], prod[:], start=True, stop=True)

        xsq_col = singles.tile([C, 1], F32)
        nc.vector.reduce_sum(xsq_col[:], xsq_part[:], axis=AX.X)
        xsq_ps = psS.tile([C, C], F32, tag="pa")
        nc.tensor.transpose(xsq_ps[:1, :], xsq_col[:], ident[:])
        # rows: s_recon, ss_recon -> mu, var -> s_row/b_row
        srow = singles.tile([1, C], F32)
        nc.vector.tensor_tensor(srow[:], sumx_row[:], r12_ps[:, :C],
                                ALU.add)
        ssrow = singles.tile([1, C], F32)
        nc.vector.tensor_scalar(ssrow[:], r3_ps[:], 2.0, None, ALU.mult)
        nc.vector.tensor_tensor(ssrow[:], ssrow[:], r12_ps[:, C:], ALU.add)
        nc.vector.tensor_tensor(ssrow[:], ssrow[:], xsq_ps[:1, :], ALU.add)
        mu_row = singles.tile([1, C], F32)
        nc.vector.tensor_scalar(mu_row[:], srow[:], 1.0 / N, None, ALU.mult)
        var_row = singles.tile([1, C], F32)
        musq_row = singles.tile([1, C], F32)
        nc.scalar.activation(musq_row[:], mu_row[:], AF.Square)
        nc.vector.tensor_scalar(var_row[:], ssrow[:], 1.0 / N, None, ALU.mult)
        nc.vector.tensor_tensor(var_row[:], var_row[:], musq_row[:],
                                ALU.subtract)
        vpe = singles.tile([1, C], F32)
        nc.vector.tensor_scalar(vpe[:], var_row[:], 1e-5, None, ALU.add)
        inv = singles.tile([1, C], F32)
        nc.vector.reciprocal(inv[:], vpe[:])
        rs_i = singles.tile([1, C], F32)
        nc.scalar.activation(rs_i[:], vpe[:], AF.Ln)
        nc.scalar.activation(rs_i[:], rs_i[:], AF.Exp, scale=-0.5)
        rbn = singles.tile([1, C], F32)          # rsqrt(var/(var+eps)+1e-5)
        nc.vector.tensor_tensor(rbn[:], var_row[:], inv[:], ALU.mult)
        nc.vector.tensor_scalar(rbn[:], rbn[:], 1e-5, None, ALU.add)
        nc.scalar.activation(rbn[:], rbn[:], AF.Ln)
        nc.scalar.activation(rbn[:], rbn[:], AF.Exp, scale=-0.5)
        s_rowt = singles.tile([1, C], F32)
        b_rowt = singles.tile([1, C], F32)
        nc.vector.tensor_tensor(s_rowt[:], rs_i[:], rbn[:], ALU.mult)
        nc.vector.tensor_tensor(s_rowt[:], s_rowt[:], bnw_row[:], ALU.mult)
        nc.vector.tensor_tensor(b_rowt[:], mu_row[:], s_rowt[:], ALU.mult)
        nc.vector.tensor_tensor(b_rowt[:], bnb_row[:], b_rowt[:],
                                ALU.subtract)
        s_ps = psD.tile([C, 1], F32, tag="pd2")
        nc.tensor.transpose(s_ps[:], s_rowt[:], ident[:1, :1])
        b_ps = psD.tile([C, 1], F32, tag="pd1")
        nc.tensor.transpose(b_ps[:], b_rowt[:], ident[:1, :1])
        sb_col = singles.tile([C, 2], F32)
        nc.vector.tensor_copy(sb_col[:, 0:1], s_ps[:])
        nc.vector.tensor_copy(sb_col[:, 1:2], b_ps[:])
        s_col, b_col = sb_col[:, 0:1], sb_col[:, 1:2]

        # ---------------- pass R: scatter(+x) matmul + gelu-sum ------------
        # recon = x + featT.T @ Mrow computed fully in PSUM (identity matmul
        # accumulates x); scalar engine does gelu straight from PSUM with the
        # per-channel affine (s,b) and accumulates the SE mean.
        gsum_part = singles.tile([C, N // CH], F32)
        gt = []
        for r in range(N // CH):
            off = r * CH
            sc_ps = psR.tile([C, CH], F32, tag="pr")
            nc.tensor.matmul(sc_ps[:], featT2_bf[:], Mrow[:, off:off + CH],
                             start=True, stop=False)
            nc.tensor.matmul(sc_ps[:], ident_bf[:], xsl(off, CH),
                             start=False, stop=True)
            g = gpool.tile([C, CH], BF16, tag=f"g{r}")
            nc.scalar.activation(g[:], sc_ps[:], AF.Gelu, bias=b_col,
                                 scale=s_col, accum_out=gsum_part[:, r:r + 1])
            gt.append(g)

        # ---------------- SE gates ----------------
        gsum_col = singles.tile([C, 1], F32)
        nc.vector.reduce_sum(gsum_col[:], gsum_part[:], axis=AX.X)
        sq_ps = psD.tile([C, 1], F32, tag="pd2")
        nc.tensor.matmul(sq_ps[:], convwT[:], gsum_col[:], start=True,
                         stop=True)
        sq = singles.tile([C, 1], F32)
        nc.vector.tensor_scalar(sq[:], sq_ps[:], 1.0 / N, convb_c,
                                ALU.mult, ALU.add)
        f1_ps = psD.tile([C // 2, 1], F32, tag="pd1")
        nc.tensor.matmul(f1_ps[:], fc1wT[:], sq[:], start=True, stop=True)
        f1 = singles.tile([C // 2, 1], F32)
        nc.scalar.activation(f1[:], f1_ps[:], AF.Gelu, bias=fc1b_c)
        f2_ps = psD.tile([C, 1], F32, tag="pd2")
        nc.tensor.matmul(f2_ps[:], fc2wT[:], f1[:], start=True, stop=True)
        # sigmoid(z) = 0.5 + 0.5*tanh(z/2) -- stays on the gelu table set
        f2 = singles.tile([C, 1], F32)
        nc.scalar.activation(f2[:], f2_ps[:], AF.Tanh, scale=0.5,
                             bias=fc2b_half[:])
        nc.vector.tensor_scalar(f2[:], f2[:], 0.5, 0.5, ALU.mult, ALU.add)
        fb = singles.tile([C, 1], F32)     # f2 * conv0_b
        nc.vector.tensor_tensor(fb[:], f2[:], convb_c, ALU.mult)

        # ---------------- pass F: conv + gate + store ----------------
        for r in range(N // CH):
            off = r * CH
            cv_ps = psR.tile([C, CH], F32, tag="pr")
            nc.tensor.matmul(cv_ps[:], convwT_bf[:], gt[r][:], start=True,
                             stop=True)
            ot = och.tile([C, CH], F32, tag="ot")
            nc.scalar.activation(ot[:], cv_ps[:], AF.Identity, bias=fb[:],
                                 scale=f2[:])
            nc.sync.dma_start(out_d.ap()[:, off:off + CH], ot[:])


_NC_CACHE = {}


def _get_nc():
    if "nc" not in _NC_CACHE:
        _NC_CACHE["nc"] = build_nc()
    return _NC_CACHE["nc"]


def kernel(**inputs):
    x = np.ascontiguousarray(np.asarray(inputs["x"], dtype=np.float32))
    logits = np.ascontiguousarray(np.asarray(inputs["logits"],
                                             dtype=np.float32))
    assert x.shape == (B, C, N, 1) and logits.shape == (B, N)
    ident = np.eye(C, dtype=np.float32)
    shared = {"ident": ident}
    for nm in ("Wq1", "Wk1", "Wv1", "Wq2", "Wk2", "Wv2", "Wq3", "Wk3", "Wv3",
               "conv0_w", "fc1_w", "fc2_w", "ln_w", "ln_b", "bn_w", "bn_b",
               "conv0_b", "fc1_b", "fc2_b"):
        shared[nm] = np.ascontiguousarray(np.asarray(inputs[nm],
                                                     dtype=np.float32))
    in_maps = []
    for i in range(NCORES):
        m = dict(shared)
        m["x"] = np.ascontiguousarray(x[i, :, :, 0])
        m["logits"] = np.ascontiguousarray(logits[i])
        in_maps.append(m)

    nc = _get_nc()
    res = run_bass_kernel_spmd(nc, in_maps, list(range(NCORES))).results
    out = np.stack([res[i]["out"] for i in range(NCORES)], axis=0)
    return out[..., None].astype(np.float32)


# revision 8
# speedup vs baseline: 1.4898x; 1.0297x over previous
"""Trainium2 Bass kernel for nn_GSA_74045236183284 (histogram_binning).

Sharding: data-parallel over batch B=8 across 8 NeuronCores (1 sample/core).
All params replicated. Zero collectives: BatchNorm batch-variance is
approximated by the local sample's var/(var+eps) (deviation <3e-6 rel).
InstanceNorm statistics are computed in closed form from bin sums/counts and
sum(x)/sum(x^2), avoiding extra passes over the 8MB stream.

v3 notes:
 - x loaded via SWDGE casting DMA straight to bf16; all streaming matmuls
   (transpose, pool, scatter, conv) run bf16 with fast weight load.
 - chunk transposes are regular matmuls against a bf16 identity.
 - recon add (x + scatter) folded into the scatter matmul as an accumulating
   identity matmul -> PSUM; scalar gelu reads PSUM directly.
 - exactly 2 activation table loads: exp_and_others for the front half
   (tanh/square/exp), gelu_and_others for the back half (gelu/tanh/identity).
   Every rsqrt is a DVE Newton iteration (int32 bit-trick seed), sigmoid is
   0.5+0.5*tanh(x/2).
 - masks built directly in bf16 so the Mrow DRAM roundtrip runs on HWDGE.
"""

import sys

for _p in ("/opt/trn_rl_repo",):
    if _p not in sys.path:
        sys.path.insert(0, _p)

import numpy as np

import concourse.bass as bass
import concourse.bacc as bacc
import concourse.mybir as mybir
import concourse.tile as tile
from concourse.bass_utils import run_bass_kernel_spmd

F32 = mybir.dt.float32
BF16 = mybir.dt.bfloat16
I32 = mybir.dt.int32
AF = mybir.ActivationFunctionType
ALU = mybir.AluOpType
AX = mybir.AxisListType

B, C, N, K = 8, 128, 16384, 8
NCORES = 8
LOADCH = 2048   # x cast-load chunk (8 chunks)
CH = 512        # scatter/conv chunk (32 chunks)
MAGIC = 0x5F3759DF


def build_nc():
    nc = bacc.Bacc("TRN2", target_bir_lowering=False, debug=False,
                   num_devices=NCORES)

    x_d = nc.dram_tensor("x", [C, N], F32, kind="ExternalInput")
    logits_d = nc.dram_tensor("logits", [N], F32, kind="ExternalInput")
    ident_d = nc.dram_tensor("ident", [C, C], F32, kind="ExternalInput")
    w_d = {}
    for nm in ("Wq1", "Wk1", "Wv1", "Wq2", "Wk2", "Wv2", "Wq3", "Wk3", "Wv3",
               "conv0_w"):
        w_d[nm] = nc.dram_tensor(nm, [C, C], F32, kind="ExternalInput")
    fc1w_d = nc.dram_tensor("fc1_w", [C // 2, C], F32, kind="ExternalInput")
    fc2w_d = nc.dram_tensor("fc2_w", [C, C // 2], F32, kind="ExternalInput")
    vecs = {}
    for nm in ("ln_w", "ln_b", "bn_w", "bn_b", "conv0_b", "fc2_b"):
        vecs[nm] = nc.dram_tensor(nm, [C], F32, kind="ExternalInput")
    vecs["fc1_b"] = nc.dram_tensor("fc1_b", [C // 2], F32, kind="ExternalInput")
    out_d = nc.dram_tensor("out", [C, N], F32, kind="ExternalOutput")

    with tile.TileContext(nc) as tc:
        _body(tc, nc, x_d, logits_d, ident_d, w_d, fc1w_d, fc2w_d, vecs, out_d)

    nc.compile()
    return nc


def _body(tc, nc, x_d, logits_d, ident_d, w_d, fc1w_d, fc2w_d, vecs, out_d):
    from contextlib import ExitStack
    ctx = ExitStack()
    with ctx:
        singles = ctx.enter_context(tc.tile_pool(name="singles", bufs=1))
        xpool = ctx.enter_context(tc.tile_pool(name="xpool", bufs=1))
        gpool = ctx.enter_context(tc.tile_pool(name="gpool", bufs=1))
        xtp = ctx.enter_context(tc.tile_pool(name="xtp", bufs=3))
        och = ctx.enter_context(tc.tile_pool(name="och", bufs=3))
        dramp = ctx.enter_context(tc.tile_pool(name="dramp", bufs=1, space="DRAM"))
        psT = ctx.enter_context(tc.tile_pool(name="psT", bufs=2, space="PSUM"))
        psS = ctx.enter_context(tc.tile_pool(name="psS", bufs=1, space="PSUM"))
        psB = ctx.enter_context(tc.tile_pool(name="psB", bufs=1, space="PSUM"))
        psD = ctx.enter_context(tc.tile_pool(name="psD", bufs=1, space="PSUM"))
        psR = ctx.enter_context(tc.tile_pool(name="psR", bufs=3, space="PSUM"))

        # ---------------- DVE Newton rsqrt helper (no ACT table) -----------
        def rsqrt_inplace(row, width, tag, iters=2):
            """row[:1,:width] := 1/sqrt(row) via int bit-trick + Newton."""
            iv = singles.tile([1, width], I32, tag=f"nw_i{tag}")
            nc.vector.tensor_copy(iv[:], row.bitcast(I32))
            nc.vector.tensor_scalar(iv[:], iv[:], 1, None,
                                    ALU.logical_shift_right)
            mg = singles.tile([1, width], I32, tag=f"nw_m{tag}")
            nc.vector.memset(mg[:], MAGIC)
            nc.vector.tensor_tensor(iv[:], mg[:], iv[:], ALU.subtract)
            y = singles.tile([1, width], F32, tag=f"nw_y{tag}")
            nc.vector.tensor_copy(y[:], iv[:].bitcast(F32))
            t = singles.tile([1, width], F32, tag=f"nw_t{tag}")
            for _ in range(iters):
                nc.vector.tensor_tensor(t[:], y[:], y[:], ALU.mult)
                nc.vector.tensor_tensor(t[:], t[:], row, ALU.mult)
                nc.vector.tensor_scalar(t[:], t[:], -0.5, 1.5,
                                        ALU.mult, ALU.add)
                nc.vector.tensor_tensor(y[:], y[:], t[:], ALU.mult)
            nc.vector.tensor_copy(row, y[:])

        # ---------------- constants / small loads ----------------
        ident = singles.tile([C, C], F32)
        nc.sync.dma_start(ident[:], ident_d.ap())
        ident_bf = singles.tile([C, C], BF16)
        nc.vector.tensor_copy(ident_bf[:], ident[:])
        ones_col = singles.tile([C, 1], F32)
        nc.vector.memset(ones_col[:], 1.0)
        ones_row = singles.tile([1, C], F32)
        nc.vector.memset(ones_row[:], 1.0)
        ones8 = singles.tile([K, 1], F32)
        nc.vector.memset(ones8[:], 1.0)
        ones9 = singles.tile([K + 1, 1], F32)
        nc.vector.memset(ones9[:], 1.0)
        neg8 = singles.tile([K, 1], F32)
        nc.vector.memset(neg8[:], -1.0)

        lg = singles.tile([C, C], F32)   # logits as [p, f], n = p*128+f
        nc.sync.dma_start(lg[:], logits_d.ap().rearrange("(p f) -> p f", f=C))

        # x: 8 chunk tiles of [128, 2048] bf16 via casting SWDGE DMA
        xt = []
        for ci in range(N // LOADCH):
            t = xpool.tile([C, LOADCH], BF16, tag=f"x{ci}")
            nc.gpsimd.dma_start(t[:], x_d.ap()[:, ci * LOADCH:(ci + 1) * LOADCH])
            xt.append(t)

        def xsl(off, width):
            ci, sub = divmod(off, LOADCH)
            return xt[ci][:, sub:sub + width]

        # weight loads + transposes (WqT|WkT packed per layer; 1/temp folded
        # into WqT at prep time)
        temp = float(np.sqrt(np.float32(C)))
        wsb = {}
        for nm in w_d:
            t = singles.tile([C, C], F32, tag=f"wl_{nm}")
            nc.sync.dma_start(t[:], w_d[nm].ap())
            wsb[nm] = t
        fc1w = singles.tile([C // 2, C], F32)
        nc.sync.dma_start(fc1w[:], fc1w_d.ap())
        fc2w = singles.tile([C, C // 2], F32)
        nc.sync.dma_start(fc2w[:], fc2w_d.ap())

        wqkT = []
        wvT = []
        for l in range(3):
            qk = singles.tile([C, 2 * C], F32, tag=f"wqkT{l}")
            for s, nm in enumerate((f"Wq{l+1}", f"Wk{l+1}")):
                ps = psS.tile([C, C], F32, tag="pa")
                nc.tensor.transpose(ps[:], wsb[nm][:], ident[:])
                nc.scalar.activation(qk[:, s * C:(s + 1) * C], ps[:], AF.Copy,
                                     scale=(1.0 / temp if s == 0 else 1.0))
            wqkT.append(qk)
            vt = singles.tile([C, C], F32, tag=f"wvT{l}")
            ps = psS.tile([C, C], F32, tag="pa")
            nc.tensor.transpose(ps[:], wsb[f"Wv{l+1}"][:], ident[:])
            nc.scalar.copy(vt[:], ps[:])
            wvT.append(vt)
        convwT = singles.tile([C, C], F32)
        ps = psS.tile([C, C], F32, tag="pa")
        nc.tensor.transpose(ps[:], wsb["conv0_w"][:], ident[:])
        nc.scalar.copy(convwT[:], ps[:])
        convwT_bf = singles.tile([C, C], BF16)
        nc.vector.tensor_copy(convwT_bf[:], convwT[:])
        fc1wT = singles.tile([C, C // 2], F32)
        ps = psS.tile([C, C], F32, tag="pa")
        nc.tensor.transpose(ps[:, :C // 2], fc1w[:], ident[:C // 2, :C // 2])
        nc.scalar.copy(fc1wT[:], ps[:, :C // 2])
        fc2wT = singles.tile([C // 2, C], F32)
        ps = psS.tile([C, C], F32, tag="pa")
        nc.tensor.transpose(ps[:C // 2, :], fc2w[:], ident[:])
        nc.scalar.copy(fc2wT[:], ps[:C // 2, :])

        # vectors: bn_w/bn_b as rows; ln_w/ln_b/conv0_b/fc2_b/fc1_b -> cols
        bnw_row = singles.tile([1, C], F32)
        nc.gpsimd.dma_start(bnw_row[:], vecs["bn_w"].ap()[None, :])
        bnb_row = singles.tile([1, C], F32)
        nc.gpsimd.dma_start(bnb_row[:], vecs["bn_b"].ap()[None, :])
        vrows = singles.tile([5, C], F32)
        nc.vector.memset(vrows[:], 0.0)
        for r, nm in enumerate(("ln_w", "ln_b", "conv0_b", "fc2_b")):
            nc.gpsimd.dma_start(vrows[r:r + 1, :], vecs[nm].ap()[None, :])
        nc.gpsimd.dma_start(vrows[4:5, :C // 2], vecs["fc1_b"].ap()[None, :])
        ps = psS.tile([C, C], F32, tag="pa")
        nc.tensor.transpose(ps[:, :5], vrows[:], ident[:5, :5])
        vcols = singles.tile([C, 5], F32)
        nc.scalar.copy(vcols[:], ps[:, :5])
        lnw_c, lnb_c = vcols[:, 0:1], vcols[:, 1:2]
        convb_c, fc2b_c = vcols[:, 2:3], vcols[:, 3:4]
        fc1b_c = vcols[:C // 2, 4:5]
        fc2b_half = singles.tile([C, 1], F32)
        nc.vector.tensor_scalar(fc2b_half[:], fc2b_c, 0.5, None, ALU.mult)

        # ---------------- masks (built in bf16; 0/1 exact) ----------------
        wA = singles.tile([C, C], F32)
        nc.scalar.activation(wA[:], lg[:], AF.Tanh)

        def build_masks(dst, src, nbins):
            # dst[:, j*128:(j+1)*128] = mask_j computed from src [128,128]
            for j in range(8):
                lo = -1.0 + 0.25 * j
                nc.vector.tensor_scalar(dst[:, j * C:(j + 1) * C], src[:],
                                        float(lo), None, ALU.is_gt)
            for j in range(7):
                nc.vector.tensor_tensor(dst[:, j * C:(j + 1) * C],
                                        dst[:, j * C:(j + 1) * C],
                                        dst[:, (j + 1) * C:(j + 2) * C],
                                        ALU.subtract)
            neq = singles.tile([C, C], BF16, tag=f"neq{nbins}")
            nc.vector.tensor_scalar(neq[:], src[:], 0.0, None, ALU.not_equal)
            nc.vector.tensor_tensor(dst[:, 3 * C:4 * C], dst[:, 3 * C:4 * C],
                                    neq[:], ALU.mult)
            if nbins > 8:
                nc.vector.memset(dst[:, 8 * C:9 * C], 1.0)

        mA = singles.tile([C, 8 * C], BF16)    # A-layout: [p, j*128+f]
        build_masks(mA, wA, 8)

        # Mrow (bf16) via DRAM roundtrip on HWDGE: mrow[j, n], n = p*128+f
        mrow_dram = dramp.tile([K, N], BF16)
        for j in range(K):
            nc.sync.dma_start(mrow_dram[j:j + 1, :].rearrange("o n -> (o n)"),
                              mA[:, j * C:(j + 1) * C])
        Mrow = singles.tile([K, N], BF16)
        nc.sync.dma_start(Mrow[:], mrow_dram[:])

        # nums: per-bin counts. numsA[p, j] = sum_f mA[p, j*128+f]
        numsA = singles.tile([C, K], F32)
        for j in range(K):
            nc.vector.reduce_sum(numsA[:, j:j + 1], mA[:, j * C:(j + 1) * C],
                                 axis=AX.X)
        nums_ps = psD.tile([K, 1], F32, tag="pd")
        nc.tensor.matmul(nums_ps[:], numsA[:], ones_col[:], start=True,
                         stop=True)
        nums_c = singles.tile([K, 1], F32)   # counts, col [j, 1]
        nc.vector.tensor_copy(nums_c[:], nums_ps[:])
        rnums_c = singles.tile([K, 1], F32)
        nc.vector.tensor_scalar(rnums_c[:], nums_c[:], 1.0, None, ALU.max)
        nc.vector.reciprocal(rnums_c[:], rnums_c[:])

        # T-layout masks from wT (pool stationary), with ones column block
        wT_ps = psS.tile([C, C], F32, tag="pa")
        nc.tensor.transpose(wT_ps[:], wA[:], ident[:])
        wT = singles.tile([C, C], F32)
        nc.scalar.copy(wT[:], wT_ps[:])
        mT = singles.tile([C, 9 * C], BF16)    # [f, j*128 + p]; j=8 -> ones
        build_masks(mT, wT, 9)

        # ---------------- x sumsq (bf16 stream, scalar engine) -------------
        xsq_part = singles.tile([C, N // LOADCH], F32)
        for ci in range(N // LOADCH):
            scr = xtp.tile([C, LOADCH], BF16, tag="sqscr")
            nc.scalar.activation(scr[:], xt[ci][:], AF.Square,
                                 accum_out=xsq_part[:, ci:ci + 1])

        # ---------------- pooled: bf16 transposes + accumulating matmuls ---
        # pooledT[j, c] (j=8 row = sum_x) accumulated over 128 chunks of n.
        pooledT_ps = psB.tile([K + 1, C], F32)
        NGR = 4  # chunks per transpose group
        for g in range(C // NGR):
            tp = psT.tile([C, NGR * C], F32, tag="pt")
            for i in range(NGR):
                q = g * NGR + i
                nc.tensor.matmul(tp[:, i * C:(i + 1) * C],
                                 xsl(q * C, C), ident_bf[:],
                                 start=True, stop=True)
            xT = xtp.tile([C, NGR * C], BF16, tag="xT")
            if g % 4 == 3:
                nc.scalar.copy(xT[:], tp[:])
            else:
                nc.vector.tensor_copy(xT[:], tp[:])
            for i in range(NGR):
                q = g * NGR + i
                nc.tensor.matmul(pooledT_ps[:], mT[:, q::C],
                                 xT[:, i * C:(i + 1) * C],
                                 start=(q == 0), stop=(q == C - 1))
        pooledT9 = singles.tile([K + 1, C], F32)
        nc.vector.tensor_copy(pooledT9[:], pooledT_ps[:])
        pooledT = pooledT9[:K, :]
        # sumx_row = (sum of all 9 rows) - (sum of the 8 bin rows)
        s9_ps = psD.tile([1, C], F32, tag="pd")
        nc.tensor.matmul(s9_ps[:], ones9[:], pooledT9[:], start=True,
                         stop=False)
        nc.tensor.matmul(s9_ps[:], neg8[:], pooledT, start=False, stop=True)
        sumx_row = singles.tile([1, C], F32)
        nc.vector.tensor_copy(sumx_row[:], s9_ps[:])

        featT = singles.tile([K, C], F32)
        nc.vector.tensor_scalar(featT[:], pooledT, rnums_c[:], None,
                                ALU.mult)

        # ---------------- attention x3 (fea orientation [c, j]) -----------
        fea = singles.tile([C, K], F32, tag="fea0")
        fps = psS.tile([C, C], F32, tag="pa")
        nc.tensor.transpose(fps[:, :K], featT[:], ident[:K, :K])
        nc.vector.tensor_copy(fea[:], fps[:, :K])

        for l in range(3):
            qk_ps = psD.tile([K, 2 * C], F32, tag="pd")
            nc.tensor.matmul(qk_ps[:], fea[:], wqkT[l][:], start=True,
                             stop=True)
            qkT = singles.tile([K, 2 * C], F32, tag=f"qkT{l}")
            nc.vector.tensor_copy(qkT[:], qk_ps[:])
            v_ps = psS.tile([C, C], F32, tag="pa")
            nc.tensor.matmul(v_ps[:, :K], wvT[l][:], fea[:], start=True,
                             stop=True)
            vsb = singles.tile([C, K], F32, tag=f"v{l}")
            nc.vector.tensor_copy(vsb[:], v_ps[:, :K])

            at_ps = psS.tile([C, C], F32, tag="pa")
            nc.tensor.matmul(at_ps[:], qkT[:, :C], qkT[:, C:], start=True,
                             stop=True)
            esb = singles.tile([C, C], F32, tag=f"e{l}")
            sume = singles.tile([C, 1], F32, tag=f"se{l}")
            nc.scalar.activation(esb[:], at_ps[:], AF.Exp, accum_out=sume[:])
            rse = singles.tile([C, 1], F32, tag=f"rse{l}")
            nc.vector.reciprocal(rse[:], sume[:])
            eT_ps = psS.tile([C, C], F32, tag="pa")
            nc.tensor.transpose(eT_ps[:], esb[:], ident[:])
            eT = singles.tile([C, C], F32, tag=f"eT{l}")
            nc.scalar.copy(eT[:], eT_ps[:])
            ao_ps = psD.tile([C, K], F32, tag="pd")
            nc.tensor.matmul(ao_ps[:], eT[:], vsb[:], start=True, stop=True)

            osb = singles.tile([C, 2 * K], F32, tag=f"osb{l}")
            nc.vector.tensor_scalar(osb[:, :K], ao_ps[:], rse[:], None,
                                    ALU.mult)
            nc.vector.tensor_tensor(osb[:, :K], osb[:, :K], fea[:], ALU.add)
            nc.vector.tensor_tensor(osb[:, K:], osb[:, :K], osb[:, :K],
                                    ALU.mult)
            st_ps = psD.tile([1, 2 * K], F32, tag="pd")
            nc.tensor.matmul(st_ps[:], ones_col[:], osb[:], start=True,
                             stop=True)
            mr = singles.tile([1, 2 * K], F32, tag=f"mr{l}")
            nc.vector.tensor_scalar(mr[:], st_ps[:], 1.0 / C, None, ALU.mult)
            musq = singles.tile([1, K], F32, tag=f"musq{l}")
            nc.vector.tensor_tensor(musq[:], mr[:, :K], mr[:, :K], ALU.mult)
            nc.vector.tensor_tensor(mr[:, K:], mr[:, K:], musq[:],
                                    ALU.subtract)
            nc.vector.tensor_scalar(mr[:, K:], mr[:, K:], 1e-6, None, ALU.add)
            rsqrt_inplace(mr[:, K:], K, f"ln{l}")
            bc_ps = psD.tile([C, 2 * K], F32, tag="pd")
            nc.tensor.matmul(bc_ps[:], ones_row[:], mr[:], start=True,
                             stop=True)
            fea2 = singles.tile([C, K], F32, tag=f"fea{l+1}")
            nc.vector.tensor_tensor(fea2[:], osb[:, :K], bc_ps[:, :K],
                                    ALU.subtract)
            nc.vector.tensor_tensor(fea2[:], fea2[:], bc_ps[:, K:], ALU.mult)
            nc.vector.tensor_scalar(fea2[:], fea2[:], lnw_c, lnb_c,
                                    ALU.mult, ALU.add)
            fea = fea2

        # exit transpose: featT_f [j, c] (+ squared) for scatter & stats
        ftp = psS.tile([C, C], F32, tag="pa")
        nc.tensor.transpose(ftp[:K, :], fea[:], ident[:])
        featT2 = singles.tile([K, 2 * C], F32)   # [featT | featT^2]
        nc.vector.tensor_copy(featT2[:, :C], ftp[:K, :])
        nc.vector.tensor_tensor(featT2[:, C:], featT2[:, :C], featT2[:, :C],
                                ALU.mult)
        featT2_bf = singles.tile([K, C], BF16)
        nc.vector.tensor_copy(featT2_bf[:], featT2[:, :C])

        # ---------------- closed-form instance stats ----------------
        prod = singles.tile([K, C], F32)
        nc.vector.tensor_tensor(prod[:], featT2[:, :C], pooledT,
                                ALU.mult)
        r12_ps = psD.tile([1, 2 * C], F32, tag="pd")
        nc.tensor.matmul(r12_ps[:], nums_c[:], featT2[:], start=True,
                         stop=True)
        r3_ps = psS.tile([C, C], F32, tag="pa")
        nc.tensor.matmul(r3_ps[:1, :], ones8[:], prod[:], start=True,
                         stop=True)

        xsq_col = singles.tile([C, 1], F32)
        nc.vector.reduce_sum(xsq_col[:], xsq_part[:], axis=AX.X)
        # rows: s_recon, ss_recon -> mu, var -> s_row/b_row
        srow = singles.tile([1, C], F32)
        nc.vector.tensor_tensor(srow[:], sumx_row[:], r12_ps[:, :C],
                                ALU.add)
        ssrow = singles.tile([1, C], F32)
        nc.vector.tensor_scalar(ssrow[:], r3_ps[:1, :], 2.0, None, ALU.mult)
        nc.vector.tensor_tensor(ssrow[:], ssrow[:], r12_ps[:, C:], ALU.add)
        mu_row = singles.tile([1, C], F32)
        nc.vector.tensor_scalar(mu_row[:], srow[:], 1.0 / N, None, ALU.mult)
        var_row = singles.tile([1, C], F32)
        musq_row = singles.tile([1, C], F32)
        nc.vector.tensor_tensor(musq_row[:], mu_row[:], mu_row[:], ALU.mult)
        xsqr_ps = psS.tile([C, C], F32, tag="pa")
        nc.tensor.transpose(xsqr_ps[:1, :], xsq_col[:], ident[:])
        nc.vector.tensor_tensor(ssrow[:], ssrow[:], xsqr_ps[:1, :], ALU.add)
        nc.vector.tensor_scalar(var_row[:], ssrow[:], 1.0 / N, None, ALU.mult)
        nc.vector.tensor_tensor(var_row[:], var_row[:], musq_row[:],
                                ALU.subtract)
        vpe = singles.tile([1, C], F32)
        nc.vector.tensor_scalar(vpe[:], var_row[:], 1e-5, None, ALU.add)
        inv = singles.tile([1, C], F32)
        nc.vector.reciprocal(inv[:], vpe[:])
        rs_i = singles.tile([1, C], F32)
        nc.vector.tensor_copy(rs_i[:], vpe[:])
        rsqrt_inplace(rs_i[:], C, "si")
        rbn = singles.tile([1, C], F32)          # rsqrt(var/(var+eps)+1e-5)
        nc.vector.tensor_tensor(rbn[:], var_row[:], inv[:], ALU.mult)
        nc.vector.tensor_scalar(rbn[:], rbn[:], 1e-5, None, ALU.add)
        rsqrt_inplace(rbn[:], C, "sb")
        s_rowt = singles.tile([1, C], F32)
        b_rowt = singles.tile([1, C], F32)
        nc.vector.tensor_tensor(s_rowt[:], rs_i[:], rbn[:], ALU.mult)
        nc.vector.tensor_tensor(s_rowt[:], s_rowt[:], bnw_row[:], ALU.mult)
        nc.vector.tensor_tensor(b_rowt[:], mu_row[:], s_rowt[:], ALU.mult)
        nc.vector.tensor_tensor(b_rowt[:], bnb_row[:], b_rowt[:],
                                ALU.subtract)
        s_ps = psD.tile([C, 1], F32, tag="pd")
        nc.tensor.transpose(s_ps[:], s_rowt[:], ident[:1, :1])
        b_ps = psS.tile([C, C], F32, tag="pa")
        nc.tensor.transpose(b_ps[:, :1], b_rowt[:], ident[:1, :1])
        sb_col = singles.tile([C, 2], F32)
        nc.vector.tensor_copy(sb_col[:, 0:1], s_ps[:])
        nc.vector.tensor_copy(sb_col[:, 1:2], b_ps[:, :1])
        s_col, b_col = sb_col[:, 0:1], sb_col[:, 1:2]

        # ---------------- pass R: scatter(+x) matmul + gelu + DVE sum ------
        gsum_part = singles.tile([C, N // CH], F32)
        gt = []
        for r in range(N // CH):
            off = r * CH
            sc_ps = psR.tile([C, CH], F32, tag="pr")
            nc.tensor.matmul(sc_ps[:], featT2_bf[:], Mrow[:, off:off + CH],
                             start=True, stop=False)
            nc.tensor.matmul(sc_ps[:], ident_bf[:], xsl(off, CH),
                             start=False, stop=True)
            g = gpool.tile([C, CH], BF16, tag=f"g{r}")
            nc.scalar.activation(g[:], sc_ps[:], AF.Gelu, bias=b_col,
                                 scale=s_col)
            nc.vector.reduce_sum(gsum_part[:, r:r + 1], g[:], axis=AX.X)
            gt.append(g)

        # ---------------- SE gates ----------------
        gsum_col = singles.tile([C, 1], F32)
        nc.vector.reduce_sum(gsum_col[:], gsum_part[:], axis=AX.X)
        sq_ps = psD.tile([C, 1], F32, tag="pd")
        nc.tensor.matmul(sq_ps[:], convwT[:], gsum_col[:], start=True,
                         stop=True)
        sq = singles.tile([C, 1], F32)
        nc.vector.tensor_scalar(sq[:], sq_ps[:], 1.0 / N, convb_c,
                                ALU.mult, ALU.add)
        f1_ps = psD.tile([C // 2, 1], F32, tag="pd")
        nc.tensor.matmul(f1_ps[:], fc1wT[:], sq[:], start=True, stop=True)
        f1 = singles.tile([C // 2, 1], F32)
        nc.scalar.activation(f1[:], f1_ps[:], AF.Gelu, bias=fc1b_c)
        f2_ps = psD.tile([C, 1], F32, tag="pd")
        nc.tensor.matmul(f2_ps[:], fc2wT[:], f1[:], start=True, stop=True)
        # sigmoid(z) = 0.5 + 0.5*tanh(z/2) -- stays on the gelu table set
        f2 = singles.tile([C, 1], F32)
        nc.scalar.activation(f2[:], f2_ps[:], AF.Tanh, scale=0.5,
                             bias=fc2b_half[:])
        nc.vector.tensor_scalar(f2[:], f2[:], 0.5, 0.5, ALU.mult, ALU.add)
        fb = singles.tile([C, 1], F32)     # f2 * conv0_b
        nc.vector.tensor_tensor(fb[:], f2[:], convb_c, ALU.mult)

        # ---------------- pass F: conv + gate + store ----------------
        for r in range(N // CH):
            off = r * CH
            cv_ps = psR.tile([C, CH], F32, tag="pr")
            nc.tensor.matmul(cv_ps[:], convwT_bf[:], gt[r][:], start=True,
                             stop=True)
            ot = och.tile([C, CH], F32, tag="ot")
            if r % 2 == 0:
                nc.scalar.activation(ot[:], cv_ps[:], AF.Identity, bias=fb[:],
                                     scale=f2[:])
            else:
                nc.vector.tensor_scalar(ot[:], cv_ps[:], f2[:], fb[:],
                                        ALU.mult, ALU.add)
            nc.sync.dma_start(out_d.ap()[:, off:off + CH], ot[:])


_NC_CACHE = {}


def _get_nc():
    if "nc" not in _NC_CACHE:
        _NC_CACHE["nc"] = build_nc()
    return _NC_CACHE["nc"]


def kernel(**inputs):
    x = np.ascontiguousarray(np.asarray(inputs["x"], dtype=np.float32))
    logits = np.ascontiguousarray(np.asarray(inputs["logits"],
                                             dtype=np.float32))
    assert x.shape == (B, C, N, 1) and logits.shape == (B, N)
    ident = np.eye(C, dtype=np.float32)
    shared = {"ident": ident}
    for nm in ("Wq1", "Wk1", "Wv1", "Wq2", "Wk2", "Wv2", "Wq3", "Wk3", "Wv3",
               "conv0_w", "fc1_w", "fc2_w", "ln_w", "ln_b", "bn_w", "bn_b",
               "conv0_b", "fc1_b", "fc2_b"):
        shared[nm] = np.ascontiguousarray(np.asarray(inputs[nm],
                                                     dtype=np.float32))
    in_maps = []
    for i in range(NCORES):
        m = dict(shared)
        m["x"] = np.ascontiguousarray(x[i, :, :, 0])
        m["logits"] = np.ascontiguousarray(logits[i])
        in_maps.append(m)

    nc = _get_nc()
    res = run_bass_kernel_spmd(nc, in_maps, list(range(NCORES))).results
    out = np.stack([res[i]["out"] for i in range(NCORES)], axis=0)
    return out[..., None].astype(np.float32)


# revision 11
# speedup vs baseline: 1.6504x; 1.1078x over previous
"""Trainium2 Bass kernel for nn_GSA_74045236183284 (histogram_binning).

Sharding: data-parallel over batch B=8 across 8 NeuronCores (1 sample/core).
All params replicated. Zero collectives: BatchNorm batch-variance is
approximated by the local sample's var/(var+eps) (deviation <3e-6 rel).
InstanceNorm statistics are computed in closed form from bin sums/counts and
sum(x)/sum(x^2), avoiding extra passes over the 8MB stream.

v4 notes:
 - x loaded via SWDGE casting DMA straight to bf16 (loads issued first);
   mask pipeline (tanh -> transpose -> bin compare) emitted before weight
   prep so pooling can start as soon as the first chunks land.
 - all streaming matmuls (transpose, pool, scatter, conv) run bf16.
 - recon add (x + scatter) folded into the scatter matmul as an accumulating
   identity matmul -> PSUM; scalar gelu reads PSUM directly, 1024 wide.
 - exactly 2 activation table loads (exp_and_others / gelu_and_others);
   every rsqrt is a DVE Newton iteration, sigmoid is 0.5+0.5*tanh(x/2).
 - PSUM: pt[2]+pooled[1]+pd[1]+pr[2x2] = 8 banks.
"""

import sys

for _p in ("/opt/trn_rl_repo",):
    if _p not in sys.path:
        sys.path.insert(0, _p)

import numpy as np

import concourse.bass as bass
import concourse.bacc as bacc
import concourse.mybir as mybir
import concourse.tile as tile
from concourse.bass_utils import run_bass_kernel_spmd

F32 = mybir.dt.float32
BF16 = mybir.dt.bfloat16
I32 = mybir.dt.int32
AF = mybir.ActivationFunctionType
ALU = mybir.AluOpType
AX = mybir.AxisListType

B, C, N, K = 8, 128, 16384, 8
NCORES = 8
LOADCH = 2048   # x cast-load chunk (8 chunks)
RCH = 1024      # scatter/conv chunk (16 chunks)
MAGIC = 0x5F3759DF


def build_nc():
    nc = bacc.Bacc("TRN2", target_bir_lowering=False, debug=False,
                   num_devices=NCORES)

    x_d = nc.dram_tensor("x", [C, N], F32, kind="ExternalInput")
    logits_d = nc.dram_tensor("logits", [N], F32, kind="ExternalInput")
    ident_d = nc.dram_tensor("ident", [C, C], F32, kind="ExternalInput")
    w_d = {}
    for nm in ("Wq1", "Wk1", "Wv1", "Wq2", "Wk2", "Wv2", "Wq3", "Wk3", "Wv3",
               "conv0_w"):
        w_d[nm] = nc.dram_tensor(nm, [C, C], F32, kind="ExternalInput")
    fc1w_d = nc.dram_tensor("fc1_w", [C // 2, C], F32, kind="ExternalInput")
    fc2w_d = nc.dram_tensor("fc2_w", [C, C // 2], F32, kind="ExternalInput")
    vecs = {}
    for nm in ("ln_w", "ln_b", "bn_w", "bn_b", "conv0_b", "fc2_b"):
        vecs[nm] = nc.dram_tensor(nm, [C], F32, kind="ExternalInput")
    vecs["fc1_b"] = nc.dram_tensor("fc1_b", [C // 2], F32, kind="ExternalInput")
    out_d = nc.dram_tensor("out", [C, N], F32, kind="ExternalOutput")

    with tile.TileContext(nc) as tc:
        _body(tc, nc, x_d, logits_d, ident_d, w_d, fc1w_d, fc2w_d, vecs, out_d)

    nc.compile()
    return nc


def _body(tc, nc, x_d, logits_d, ident_d, w_d, fc1w_d, fc2w_d, vecs, out_d):
    from contextlib import ExitStack
    ctx = ExitStack()
    with ctx:
        singles = ctx.enter_context(tc.tile_pool(name="singles", bufs=1))
        xpool = ctx.enter_context(tc.tile_pool(name="xpool", bufs=1))
        gpool = ctx.enter_context(tc.tile_pool(name="gpool", bufs=1))
        xtp = ctx.enter_context(tc.tile_pool(name="xtp", bufs=3))
        och = ctx.enter_context(tc.tile_pool(name="och", bufs=3))
        dramp = ctx.enter_context(tc.tile_pool(name="dramp", bufs=1, space="DRAM"))
        psT = ctx.enter_context(tc.tile_pool(name="psT", bufs=2, space="PSUM"))
        psB = ctx.enter_context(tc.tile_pool(name="psB", bufs=1, space="PSUM"))
        psD = ctx.enter_context(tc.tile_pool(name="psD", bufs=1, space="PSUM"))
        psR = ctx.enter_context(tc.tile_pool(name="psR", bufs=2, space="PSUM"))

        def pt_tile():
            return psT.tile([C, 4 * C], F32, tag="pt", name="pt")

        # ---------------- DVE Newton rsqrt helper (no ACT table) -----------
        nw_scr = {}

        def rsqrt_inplace(row, width, iters=2):
            """row[:1,:width] := 1/sqrt(row) via int bit-trick + Newton."""
            if width not in nw_scr:
                iv = singles.tile([1, width], I32, tag=f"nw_i{width}",
                                  name=f"nw_i{width}")
                mg = singles.tile([1, width], I32, tag=f"nw_m{width}",
                                  name=f"nw_m{width}")
                nc.vector.memset(mg[:], MAGIC)
                y = singles.tile([1, width], F32, tag=f"nw_y{width}",
                                 name=f"nw_y{width}")
                t = singles.tile([1, width], F32, tag=f"nw_t{width}",
                                 name=f"nw_t{width}")
                nw_scr[width] = (iv, mg, y, t)
            iv, mg, y, t = nw_scr[width]
            nc.vector.tensor_copy(iv[:], row.bitcast(I32))
            nc.vector.tensor_scalar(iv[:], iv[:], 1, None,
                                    ALU.logical_shift_right)
            nc.vector.tensor_tensor(iv[:], mg[:], iv[:], ALU.subtract)
            nc.vector.tensor_copy(y[:], iv[:].bitcast(F32))
            for _ in range(iters):
                nc.vector.tensor_tensor(t[:], y[:], y[:], ALU.mult)
                nc.vector.tensor_tensor(t[:], t[:], row, ALU.mult)
                nc.vector.tensor_scalar(t[:], t[:], -0.5, 1.5,
                                        ALU.mult, ALU.add)
                nc.vector.tensor_tensor(y[:], y[:], t[:], ALU.mult)
            nc.vector.tensor_copy(row, y[:])

        # ---------------- loads first: logits, identity, x ----------------
        lg = singles.tile([C, C], F32)   # logits as [p, f], n = p*128+f
        nc.sync.dma_start(lg[:], logits_d.ap().rearrange("(p f) -> p f", f=C))
        ident = singles.tile([C, C], F32)
        nc.sync.dma_start(ident[:], ident_d.ap())
        ident_bf = singles.tile([C, C], BF16)
        nc.vector.tensor_copy(ident_bf[:], ident[:])

        xt = []
        for ci in range(N // LOADCH):
            t = xpool.tile([C, LOADCH], BF16, tag=f"x{ci}", name=f"x{ci}")
            nc.gpsimd.dma_start(t[:], x_d.ap()[:, ci * LOADCH:(ci + 1) * LOADCH])
            xt.append(t)

        def xsl(off, width):
            ci, sub = divmod(off, LOADCH)
            return xt[ci][:, sub:sub + width]

        # ---------------- masks (emitted early; built in bf16) -------------
        wA = singles.tile([C, C], F32)
        nc.scalar.activation(wA[:], lg[:], AF.Tanh)
        wT_ps = pt_tile()
        nc.tensor.transpose(wT_ps[:, :C], wA[:], ident[:])
        wT = singles.tile([C, C], F32)
        nc.scalar.copy(wT[:], wT_ps[:, :C])

        def build_masks(dst, src, nbins):
            # dst[:, j*128:(j+1)*128] = mask_j computed from src [128,128]
            for j in range(8):
                lo = -1.0 + 0.25 * j
                nc.vector.tensor_scalar(dst[:, j * C:(j + 1) * C], src[:],
                                        float(lo), None, ALU.is_gt)
            for j in range(7):
                nc.vector.tensor_tensor(dst[:, j * C:(j + 1) * C],
                                        dst[:, j * C:(j + 1) * C],
                                        dst[:, (j + 1) * C:(j + 2) * C],
                                        ALU.subtract)
            neq = singles.tile([C, C], BF16, tag=f"neq{nbins}",
                               name=f"neq{nbins}")
            nc.vector.tensor_scalar(neq[:], src[:], 0.0, None, ALU.not_equal)
            nc.vector.tensor_tensor(dst[:, 3 * C:4 * C], dst[:, 3 * C:4 * C],
                                    neq[:], ALU.mult)
            if nbins > 8:
                nc.vector.memset(dst[:, 8 * C:9 * C], 1.0)

        mT = singles.tile([C, 9 * C], BF16)    # [f, j*128 + p]; j=8 -> ones
        build_masks(mT, wT, 9)
        mA = singles.tile([C, 8 * C], BF16)    # A-layout: [p, j*128+f]
        build_masks(mA, wA, 8)

        # ---------------- pooled: bf16 transposes + accumulating matmuls ---
        # pooledT[j, c] (j=8 row = sum_x) accumulated over 128 chunks of n.
        ones_col = singles.tile([C, 1], F32)
        nc.vector.memset(ones_col[:], 1.0)
        pooledT_ps = psB.tile([K + 1, C], F32)
        NGR = 4  # chunks per transpose group
        for g in range(C // NGR):
            tp = pt_tile()
            for i in range(NGR):
                q = g * NGR + i
                nc.tensor.matmul(tp[:, i * C:(i + 1) * C],
                                 xsl(q * C, C), ident_bf[:],
                                 start=True, stop=True)
            xT = xtp.tile([C, NGR * C], BF16, tag="xT", name="xT")
            if g % 4 == 1:
                nc.scalar.copy(xT[:], tp[:])
            else:
                nc.vector.tensor_copy(xT[:], tp[:])
            for i in range(NGR):
                q = g * NGR + i
                nc.tensor.matmul(pooledT_ps[:], mT[:, q::C],
                                 xT[:, i * C:(i + 1) * C],
                                 start=(q == 0), stop=(q == C - 1))

        # ---------------- x sumsq (bf16 stream, scalar engine) -------------
        xsq_part = singles.tile([C, N // LOADCH], F32)
        for ci in range(N // LOADCH):
            scr = xtp.tile([C, LOADCH], BF16, tag="sqscr", name="sqscr")
            nc.scalar.activation(scr[:], xt[ci][:], AF.Square,
                                 accum_out=xsq_part[:, ci:ci + 1])

        # ---------------- Mrow (bf16) via HWDGE DRAM roundtrip -------------
        mrow_dram = dramp.tile([K, N], BF16)
        for j in range(K):
            nc.sync.dma_start(mrow_dram[j:j + 1, :].rearrange("o n -> (o n)"),
                              mA[:, j * C:(j + 1) * C])
        Mrow = singles.tile([K, N], BF16)
        nc.sync.dma_start(Mrow[:], mrow_dram[:])

        # nums: per-bin counts. numsA[p, j] = sum_f mA[p, j*128+f]
        numsA = singles.tile([C, K], F32)
        for j in range(K):
            nc.vector.reduce_sum(numsA[:, j:j + 1], mA[:, j * C:(j + 1) * C],
                                 axis=AX.X)
        nums_ps = psD.tile([K, 1], F32, tag="pd", name="nums_ps")
        nc.tensor.matmul(nums_ps[:], numsA[:], ones_col[:], start=True,
                         stop=True)
        nums_c = singles.tile([K, 1], F32)   # counts, col [j, 1]
        nc.vector.tensor_copy(nums_c[:], nums_ps[:])
        rnums_c = singles.tile([K, 1], F32)
        nc.vector.tensor_scalar(rnums_c[:], nums_c[:], 1.0, None, ALU.max)
        nc.vector.reciprocal(rnums_c[:], rnums_c[:])

        # ---------------- weights / vectors prep (off critical path) -------
        temp = float(np.sqrt(np.float32(C)))
        wsb = {}
        for nm in w_d:
            t = singles.tile([C, C], F32, tag=f"wl_{nm}", name=f"wl_{nm}")
            nc.sync.dma_start(t[:], w_d[nm].ap())
            wsb[nm] = t
        fc1w = singles.tile([C // 2, C], F32)
        nc.sync.dma_start(fc1w[:], fc1w_d.ap())
        fc2w = singles.tile([C, C // 2], F32)
        nc.sync.dma_start(fc2w[:], fc2w_d.ap())

        wqkT = []
        wvT = []
        for l in range(3):
            qk = singles.tile([C, 2 * C], F32, tag=f"wqkT{l}",
                              name=f"wqkT{l}")
            for s, nm in enumerate((f"Wq{l+1}", f"Wk{l+1}")):
                ps = pt_tile()
                nc.tensor.transpose(ps[:, :C], wsb[nm][:], ident[:])
                nc.scalar.activation(qk[:, s * C:(s + 1) * C], ps[:, :C],
                                     AF.Copy,
                                     scale=(1.0 / temp if s == 0 else 1.0))
            wqkT.append(qk)
            vt = singles.tile([C, C], F32, tag=f"wvT{l}", name=f"wvT{l}")
            ps = pt_tile()
            nc.tensor.transpose(ps[:, :C], wsb[f"Wv{l+1}"][:], ident[:])
            nc.scalar.copy(vt[:], ps[:, :C])
            wvT.append(vt)
        convwT = singles.tile([C, C], F32)
        ps = pt_tile()
        nc.tensor.transpose(ps[:, :C], wsb["conv0_w"][:], ident[:])
        nc.scalar.copy(convwT[:], ps[:, :C])
        convwT_bf = singles.tile([C, C], BF16)
        nc.vector.tensor_copy(convwT_bf[:], convwT[:])
        fc1wT = singles.tile([C, C // 2], F32)
        ps = pt_tile()
        nc.tensor.transpose(ps[:, :C // 2], fc1w[:], ident[:C // 2, :C // 2])
        nc.scalar.copy(fc1wT[:], ps[:, :C // 2])
        fc2wT = singles.tile([C // 2, C], F32)
        ps = pt_tile()
        nc.tensor.transpose(ps[:C // 2, :C], fc2w[:], ident[:])
        nc.scalar.copy(fc2wT[:], ps[:C // 2, :C])

        bnw_row = singles.tile([1, C], F32)
        nc.sync.dma_start(bnw_row[:], vecs["bn_w"].ap()[None, :])
        bnb_row = singles.tile([1, C], F32)
        nc.sync.dma_start(bnb_row[:], vecs["bn_b"].ap()[None, :])
        vrows = singles.tile([5, C], F32)
        nc.vector.memset(vrows[:], 0.0)
        for r, nm in enumerate(("ln_w", "ln_b", "conv0_b", "fc2_b")):
            nc.sync.dma_start(vrows[r:r + 1, :], vecs[nm].ap()[None, :])
        nc.sync.dma_start(vrows[4:5, :C // 2], vecs["fc1_b"].ap()[None, :])
        ps = pt_tile()
        nc.tensor.transpose(ps[:, :5], vrows[:], ident[:5, :5])
        vcols = singles.tile([C, 5], F32)
        nc.scalar.copy(vcols[:], ps[:, :5])
        lnw_c, lnb_c = vcols[:, 0:1], vcols[:, 1:2]
        convb_c, fc2b_c = vcols[:, 2:3], vcols[:, 3:4]
        fc1b_c = vcols[:C // 2, 4:5]
        fc2b_half = singles.tile([C, 1], F32)
        nc.vector.tensor_scalar(fc2b_half[:], fc2b_c, 0.5, None, ALU.mult)
        ones_row = singles.tile([1, C], F32)
        nc.vector.memset(ones_row[:], 1.0)
        ones8 = singles.tile([K, 1], F32)
        nc.vector.memset(ones8[:], 1.0)
        ones9 = singles.tile([K + 1, 1], F32)
        nc.vector.memset(ones9[:], 1.0)
        neg8 = singles.tile([K, 1], F32)
        nc.vector.memset(neg8[:], -1.0)

        # ---------------- pooled epilogue ----------------
        pooledT9 = singles.tile([K + 1, C], F32)
        nc.vector.tensor_copy(pooledT9[:], pooledT_ps[:])
        pooledT = pooledT9[:K, :]
        # sumx_row = (sum of all 9 rows) - (sum of the 8 bin rows)
        s9_ps = psD.tile([1, C], F32, tag="pd", name="s9_ps")
        nc.tensor.matmul(s9_ps[:], ones9[:], pooledT9[:], start=True,
                         stop=False)
        nc.tensor.matmul(s9_ps[:], neg8[:], pooledT, start=False, stop=True)
        sumx_row = singles.tile([1, C], F32)
        nc.vector.tensor_copy(sumx_row[:], s9_ps[:])

        featT = singles.tile([K, C], F32)
        nc.vector.tensor_scalar(featT[:], pooledT, rnums_c[:], None,
                                ALU.mult)

        # ---------------- attention x3 (fea orientation [c, j]) -----------
        fea = singles.tile([C, K], F32, tag="fea0", name="fea0")
        fps = pt_tile()
        nc.tensor.transpose(fps[:, :K], featT[:], ident[:K, :K])
        nc.vector.tensor_copy(fea[:], fps[:, :K])

        for l in range(3):
            qk_ps = pt_tile()
            nc.tensor.matmul(qk_ps[:K, :2 * C], fea[:], wqkT[l][:],
                             start=True, stop=True)
            qkT = singles.tile([K, 2 * C], F32, tag=f"qkT{l}",
                               name=f"qkT{l}")
            nc.vector.tensor_copy(qkT[:], qk_ps[:K, :2 * C])
            v_ps = pt_tile()
            nc.tensor.matmul(v_ps[:, :K], wvT[l][:], fea[:], start=True,
                             stop=True)
            vsb = singles.tile([C, K], F32, tag=f"v{l}", name=f"v{l}")
            nc.vector.tensor_copy(vsb[:], v_ps[:, :K])

            at_ps = pt_tile()
            nc.tensor.matmul(at_ps[:, :C], qkT[:, :C], qkT[:, C:],
                             start=True, stop=True)
            esb = singles.tile([C, C], F32, tag=f"e{l}", name=f"e{l}")
            sume = singles.tile([C, 1], F32, tag=f"se{l}", name=f"se{l}")
            nc.scalar.activation(esb[:], at_ps[:, :C], AF.Exp,
                                 accum_out=sume[:])
            rse = singles.tile([C, 1], F32, tag=f"rse{l}", name=f"rse{l}")
            nc.vector.reciprocal(rse[:], sume[:])
            eT_ps = pt_tile()
            nc.tensor.transpose(eT_ps[:, :C], esb[:], ident[:])
            eT = singles.tile([C, C], F32, tag=f"eT{l}", name=f"eT{l}")
            nc.scalar.copy(eT[:], eT_ps[:, :C])
            ao_ps = psD.tile([C, K], F32, tag="pd", name=f"ao_ps{l}")
            nc.tensor.matmul(ao_ps[:], eT[:], vsb[:], start=True, stop=True)

            osb = singles.tile([C, 2 * K], F32, tag=f"osb{l}",
                               name=f"osb{l}")
            nc.vector.tensor_scalar(osb[:, :K], ao_ps[:], rse[:], None,
                                    ALU.mult)
            nc.vector.tensor_tensor(osb[:, :K], osb[:, :K], fea[:], ALU.add)
            nc.vector.tensor_tensor(osb[:, K:], osb[:, :K], osb[:, :K],
                                    ALU.mult)
            st_ps = psD.tile([1, 2 * K], F32, tag="pd", name=f"st_ps{l}")
            nc.tensor.matmul(st_ps[:], ones_col[:], osb[:], start=True,
                             stop=True)
            mr = singles.tile([1, 2 * K], F32, tag=f"mr{l}", name=f"mr{l}")
            nc.vector.tensor_scalar(mr[:], st_ps[:], 1.0 / C, None, ALU.mult)
            musq = singles.tile([1, K], F32, tag=f"musq{l}", name=f"musq{l}")
            nc.vector.tensor_tensor(musq[:], mr[:, :K], mr[:, :K], ALU.mult)
            nc.vector.tensor_tensor(mr[:, K:], mr[:, K:], musq[:],
                                    ALU.subtract)
            nc.vector.tensor_scalar(mr[:, K:], mr[:, K:], 1e-6, None, ALU.add)
            rsqrt_inplace(mr[:, K:], K)
            bc_ps = pt_tile()
            nc.tensor.matmul(bc_ps[:, :2 * K], ones_row[:], mr[:],
                             start=True, stop=True)
            fea2 = singles.tile([C, K], F32, tag=f"fea{l+1}",
                                name=f"fea{l+1}")
            nc.vector.tensor_tensor(fea2[:], osb[:, :K], bc_ps[:, :K],
                                    ALU.subtract)
            nc.vector.tensor_tensor(fea2[:], fea2[:], bc_ps[:, K:2 * K],
                                    ALU.mult)
            nc.vector.tensor_scalar(fea2[:], fea2[:], lnw_c, lnb_c,
                                    ALU.mult, ALU.add)
            fea = fea2

        # exit transpose: featT_f [j, c] (+ squared) for scatter & stats
        ftp = pt_tile()
        nc.tensor.transpose(ftp[:K, :C], fea[:], ident[:])
        featT2 = singles.tile([K, 2 * C], F32)   # [featT | featT^2]
        nc.vector.tensor_copy(featT2[:, :C], ftp[:K, :C])
        nc.vector.tensor_tensor(featT2[:, C:], featT2[:, :C], featT2[:, :C],
                                ALU.mult)
        featT2_bf = singles.tile([K, C], BF16)
        nc.vector.tensor_copy(featT2_bf[:], featT2[:, :C])

        # ---------------- closed-form instance stats ----------------
        # packed rows: [vpe | rbn_arg] -> one Newton rsqrt over 2C
        prod = singles.tile([K, C], F32)
        nc.vector.tensor_tensor(prod[:], featT2[:, :C], pooledT,
                                ALU.mult)
        r12_ps = psD.tile([1, 2 * C], F32, tag="pd", name="r12_ps")
        nc.tensor.matmul(r12_ps[:], nums_c[:], featT2[:], start=True,
                         stop=True)
        r3_ps = pt_tile()
        nc.tensor.matmul(r3_ps[:1, :C], ones8[:], prod[:], start=True,
                         stop=True)

        xsq_col = singles.tile([C, 1], F32)
        nc.vector.reduce_sum(xsq_col[:], xsq_part[:], axis=AX.X)
        xsqr_ps = pt_tile()
        nc.tensor.transpose(xsqr_ps[:1, :C], xsq_col[:], ident[:])
        srow = singles.tile([1, C], F32)
        nc.vector.tensor_tensor(srow[:], sumx_row[:], r12_ps[:, :C],
                                ALU.add)
        ssrow = singles.tile([1, C], F32)
        nc.vector.tensor_scalar(ssrow[:], r3_ps[:1, :C], 2.0, None, ALU.mult)
        nc.vector.tensor_tensor(ssrow[:], ssrow[:], r12_ps[:, C:], ALU.add)
        nc.vector.tensor_tensor(ssrow[:], ssrow[:], xsqr_ps[:1, :C], ALU.add)
        mu_row = singles.tile([1, C], F32)
        nc.vector.tensor_scalar(mu_row[:], srow[:], 1.0 / N, None, ALU.mult)
        musq_row = singles.tile([1, C], F32)
        nc.vector.tensor_tensor(musq_row[:], mu_row[:], mu_row[:], ALU.mult)
        var_row = singles.tile([1, C], F32)
        nc.vector.tensor_scalar(var_row[:], ssrow[:], 1.0 / N, None, ALU.mult)
        nc.vector.tensor_tensor(var_row[:], var_row[:], musq_row[:],
                                ALU.subtract)
        pk = singles.tile([1, 2 * C], F32)   # [vpe | var/(var+eps)+eps]
        nc.vector.tensor_scalar(pk[:, :C], var_row[:], 1e-5, None, ALU.add)
        inv = singles.tile([1, C], F32)
        nc.vector.reciprocal(inv[:], pk[:, :C])
        nc.vector.tensor_tensor(pk[:, C:], var_row[:], inv[:], ALU.mult)
        nc.vector.tensor_scalar(pk[:, C:], pk[:, C:], 1e-5, None, ALU.add)
        rsqrt_inplace(pk[:], 2 * C)          # -> [rs_i | rs_b]
        s_rowt = singles.tile([1, C], F32)
        b_rowt = singles.tile([1, C], F32)
        nc.vector.tensor_tensor(s_rowt[:], pk[:, :C], pk[:, C:], ALU.mult)
        nc.vector.tensor_tensor(s_rowt[:], s_rowt[:], bnw_row[:], ALU.mult)
        nc.vector.tensor_tensor(b_rowt[:], mu_row[:], s_rowt[:], ALU.mult)
        nc.vector.tensor_tensor(b_rowt[:], bnb_row[:], b_rowt[:],
                                ALU.subtract)
        s_ps = psD.tile([C, 2], F32, tag="pd", name="s_ps")
        nc.tensor.transpose(s_ps[:, :1], s_rowt[:], ident[:1, :1])
        b_ps = pt_tile()
        nc.tensor.transpose(b_ps[:, :1], b_rowt[:], ident[:1, :1])
        sb_col = singles.tile([C, 2], F32)
        nc.vector.tensor_copy(sb_col[:, 0:1], s_ps[:, :1])
        nc.vector.tensor_copy(sb_col[:, 1:2], b_ps[:, :1])
        s_col, b_col = sb_col[:, 0:1], sb_col[:, 1:2]

        # ---------------- pass R: scatter(+x) matmul + gelu + DVE sum ------
        gsum_part = singles.tile([C, N // RCH], F32)
        gt = []
        for r in range(N // RCH):
            off = r * RCH
            sc_ps = psR.tile([C, RCH], F32, tag="pr", name="sc_ps")
            for h in range(2):
                sl = off + h * 512
                nc.tensor.matmul(sc_ps[:, h * 512:(h + 1) * 512],
                                 featT2_bf[:], Mrow[:, sl:sl + 512],
                                 start=True, stop=False)
                nc.tensor.matmul(sc_ps[:, h * 512:(h + 1) * 512],
                                 ident_bf[:], xsl(sl, 512),
                                 start=False, stop=True)
            g = gpool.tile([C, RCH], BF16, tag=f"g{r}", name=f"g{r}")
            nc.scalar.activation(g[:], sc_ps[:], AF.Gelu, bias=b_col,
                                 scale=s_col)
            nc.vector.reduce_sum(gsum_part[:, r:r + 1], g[:], axis=AX.X)
            gt.append(g)

        # ---------------- SE gates ----------------
        gsum_col = singles.tile([C, 1], F32)
        nc.vector.reduce_sum(gsum_col[:], gsum_part[:], axis=AX.X)
        sq_ps = psD.tile([C, 1], F32, tag="pd", name="sq_ps")
        nc.tensor.matmul(sq_ps[:], convwT[:], gsum_col[:], start=True,
                         stop=True)
        sq = singles.tile([C, 1], F32)
        nc.vector.tensor_scalar(sq[:], sq_ps[:], 1.0 / N, convb_c,
                                ALU.mult, ALU.add)
        f1_ps = psD.tile([C // 2, 1], F32, tag="pd", name="f1_ps")
        nc.tensor.matmul(f1_ps[:], fc1wT[:], sq[:], start=True, stop=True)
        f1 = singles.tile([C // 2, 1], F32)
        nc.scalar.activation(f1[:], f1_ps[:], AF.Gelu, bias=fc1b_c)
        f2_ps = psD.tile([C, 1], F32, tag="pd", name="f2_ps")
        nc.tensor.matmul(f2_ps[:], fc2wT[:], f1[:], start=True, stop=True)
        # sigmoid(z) = 0.5 + 0.5*tanh(z/2) -- stays on the gelu table set
        f2 = singles.tile([C, 1], F32)
        nc.scalar.activation(f2[:], f2_ps[:], AF.Tanh, scale=0.5,
                             bias=fc2b_half[:])
        nc.vector.tensor_scalar(f2[:], f2[:], 0.5, 0.5, ALU.mult, ALU.add)
        fb = singles.tile([C, 1], F32)     # f2 * conv0_b
        nc.vector.tensor_tensor(fb[:], f2[:], convb_c, ALU.mult)

        # ---------------- pass F: conv + gate + store ----------------
        for r in range(N // RCH):
            off = r * RCH
            cv_ps = psR.tile([C, RCH], F32, tag="pr", name="cv_ps")
            for h in range(2):
                nc.tensor.matmul(cv_ps[:, h * 512:(h + 1) * 512],
                                 convwT_bf[:], gt[r][:, h * 512:(h + 1) * 512],
                                 start=True, stop=True)
            ot = och.tile([C, RCH], F32, tag="ot", name="ot")
            if r % 2 == 0:
                nc.scalar.activation(ot[:], cv_ps[:], AF.Identity, bias=fb[:],
                                     scale=f2[:])
            else:
                nc.vector.tensor_scalar(ot[:], cv_ps[:], f2[:], fb[:],
                                        ALU.mult, ALU.add)
            nc.sync.dma_start(out_d.ap()[:, off:off + RCH], ot[:])


_NC_CACHE = {}


def _get_nc():
    if "nc" not in _NC_CACHE:
        _NC_CACHE["nc"] = build_nc()
    return _NC_CACHE["nc"]


def kernel(**inputs):
    x = np.ascontiguousarray(np.asarray(inputs["x"], dtype=np.float32))
    logits = np.ascontiguousarray(np.asarray(inputs["logits"],
                                             dtype=np.float32))
    assert x.shape == (B, C, N, 1) and logits.shape == (B, N)
    ident = np.eye(C, dtype=np.float32)
    shared = {"ident": ident}
    for nm in ("Wq1", "Wk1", "Wv1", "Wq2", "Wk2", "Wv2", "Wq3", "Wk3", "Wv3",
               "conv0_w", "fc1_w", "fc2_w", "ln_w", "ln_b", "bn_w", "bn_b",
               "conv0_b", "fc1_b", "fc2_b"):
        shared[nm] = np.ascontiguousarray(np.asarray(inputs[nm],
                                                     dtype=np.float32))
    in_maps = []
    for i in range(NCORES):
        m = dict(shared)
        m["x"] = np.ascontiguousarray(x[i, :, :, 0])
        m["logits"] = np.ascontiguousarray(logits[i])
        in_maps.append(m)

    nc = _get_nc()
    res = run_bass_kernel_spmd(nc, in_maps, list(range(NCORES))).results
    out = np.stack([res[i]["out"] for i in range(NCORES)], axis=0)
    return out[..., None].astype(np.float32)


# revision 16
# speedup vs baseline: 1.7458x; 1.0578x over previous
"""Trainium2 Bass kernel for nn_GSA_74045236183284 (histogram_binning).

Sharding: data-parallel over batch B=8 across 8 NeuronCores (1 sample/core).
All params replicated. Zero collectives: BatchNorm batch-variance is
approximated by the local sample's var/(var+eps) (deviation <3e-6 rel).
InstanceNorm statistics are computed in closed form from bin sums/counts and
sum(x)/sum(x^2), avoiding extra passes over the 8MB stream.

v4 notes:
 - x loaded via SWDGE casting DMA straight to bf16 (loads issued first);
   mask pipeline (tanh -> transpose -> bin compare) emitted before weight
   prep so pooling can start as soon as the first chunks land.
 - all streaming matmuls (transpose, pool, scatter, conv) run bf16.
 - recon add (x + scatter) folded into the scatter matmul as an accumulating
   identity matmul -> PSUM; scalar gelu reads PSUM directly, 1024 wide.
 - exactly 2 activation table loads (exp_and_others / gelu_and_others);
   every rsqrt is a DVE Newton iteration, sigmoid is 0.5+0.5*tanh(x/2).
 - PSUM: pt[2]+pooled[1]+pd[1]+pr[2x2] = 8 banks.
"""

import sys

for _p in ("/opt/trn_rl_repo",):
    if _p not in sys.path:
        sys.path.insert(0, _p)

import numpy as np

import concourse.bass as bass
import concourse.bacc as bacc
import concourse.mybir as mybir
import concourse.tile as tile
from concourse.bass_utils import run_bass_kernel_spmd

F32 = mybir.dt.float32
BF16 = mybir.dt.bfloat16
I32 = mybir.dt.int32
AF = mybir.ActivationFunctionType
ALU = mybir.AluOpType
AX = mybir.AxisListType

B, C, N, K = 8, 128, 16384, 8
NCORES = 8
LOADCH = 2048   # x cast-load chunk (8 chunks)
RCH = 1024      # scatter/conv chunk (16 chunks)
MAGIC = 0x5F3759DF


def build_nc():
    nc = bacc.Bacc("TRN2", target_bir_lowering=False, debug=False,
                   num_devices=NCORES)

    x_d = nc.dram_tensor("x", [C, N], F32, kind="ExternalInput")
    logits_d = nc.dram_tensor("logits", [N], F32, kind="ExternalInput")
    ident_d = nc.dram_tensor("ident", [C, C], F32, kind="ExternalInput")
    w_d = {}
    for nm in ("Wq1", "Wk1", "Wv1", "Wq2", "Wk2", "Wv2", "Wq3", "Wk3", "Wv3",
               "conv0_w"):
        w_d[nm] = nc.dram_tensor(nm, [C, C], F32, kind="ExternalInput")
    fc1w_d = nc.dram_tensor("fc1_w", [C // 2, C], F32, kind="ExternalInput")
    fc2w_d = nc.dram_tensor("fc2_w", [C, C // 2], F32, kind="ExternalInput")
    vecs = {}
    for nm in ("ln_w", "ln_b", "bn_w", "bn_b", "conv0_b", "fc2_b"):
        vecs[nm] = nc.dram_tensor(nm, [C], F32, kind="ExternalInput")
    vecs["fc1_b"] = nc.dram_tensor("fc1_b", [C // 2], F32, kind="ExternalInput")
    out_d = nc.dram_tensor("out", [C, N], F32, kind="ExternalOutput")

    with tile.TileContext(nc) as tc:
        _body(tc, nc, x_d, logits_d, ident_d, w_d, fc1w_d, fc2w_d, vecs, out_d)

    nc.compile()
    return nc


def _body(tc, nc, x_d, logits_d, ident_d, w_d, fc1w_d, fc2w_d, vecs, out_d):
    from contextlib import ExitStack
    ctx = ExitStack()
    with ctx:
        singles = ctx.enter_context(tc.tile_pool(name="singles", bufs=1))
        xpool = ctx.enter_context(tc.tile_pool(name="xpool", bufs=1))
        gpool = ctx.enter_context(tc.tile_pool(name="gpool", bufs=1))
        xtp = ctx.enter_context(tc.tile_pool(name="xtp", bufs=3))
        och = ctx.enter_context(tc.tile_pool(name="och", bufs=3))
        dramp = ctx.enter_context(tc.tile_pool(name="dramp", bufs=1, space="DRAM"))
        psT = ctx.enter_context(tc.tile_pool(name="psT", bufs=2, space="PSUM"))
        psB = ctx.enter_context(tc.tile_pool(name="psB", bufs=1, space="PSUM"))
        psD = ctx.enter_context(tc.tile_pool(name="psD", bufs=1, space="PSUM"))
        psR = ctx.enter_context(tc.tile_pool(name="psR", bufs=2, space="PSUM"))

        def pt_tile():
            return psT.tile([C, 4 * C], F32, tag="pt", name="pt")

        # ---------------- DVE Newton rsqrt helper (no ACT table) -----------
        nw_scr = {}

        def rsqrt_inplace(row, width, iters=2):
            """row[:1,:width] := 1/sqrt(row) via int bit-trick + Newton.
            iters=1 -> ~0.2% rel err, iters=2 -> ~5e-6."""
            if width not in nw_scr:
                iv = singles.tile([1, width], I32, tag=f"nw_i{width}",
                                  name=f"nw_i{width}")
                mg = singles.tile([1, width], I32, tag=f"nw_m{width}",
                                  name=f"nw_m{width}")
                nc.vector.memset(mg[:], MAGIC)
                t = singles.tile([1, width], F32, tag=f"nw_t{width}",
                                 name=f"nw_t{width}")
                nw_scr[width] = (iv, mg, t)
            iv, mg, t = nw_scr[width]
            nc.vector.tensor_scalar(iv[:], row.bitcast(I32), 1, None,
                                    ALU.logical_shift_right)
            nc.vector.tensor_tensor(iv[:], mg[:], iv[:], ALU.subtract)
            y = iv[:].bitcast(F32)
            for it in range(iters):
                nc.vector.tensor_tensor(t[:], y, y, ALU.mult)
                nc.vector.tensor_tensor(t[:], t[:], row, ALU.mult)
                nc.vector.tensor_scalar(t[:], t[:], -0.5, 1.5,
                                        ALU.mult, ALU.add)
                if it == iters - 1:
                    nc.vector.tensor_tensor(row, y, t[:], ALU.mult)
                else:
                    nc.vector.tensor_tensor(iv[:].bitcast(F32), y, t[:],
                                            ALU.mult)

        # ---------------- loads first: logits, identity, x ----------------
        lg = singles.tile([C, C], F32)   # logits as [p, f], n = p*128+f
        nc.sync.dma_start(lg[:], logits_d.ap().rearrange("(p f) -> p f", f=C))
        ident = singles.tile([C, C], F32)
        nc.sync.dma_start(ident[:], ident_d.ap())
        ident_bf = singles.tile([C, C], BF16)
        nc.vector.tensor_copy(ident_bf[:], ident[:])

        xt = []
        for ci in range(N // LOADCH):
            t = xpool.tile([C, LOADCH], BF16, tag=f"x{ci}", name=f"x{ci}")
            nc.gpsimd.dma_start(t[:], x_d.ap()[:, ci * LOADCH:(ci + 1) * LOADCH])
            xt.append(t)

        def xsl(off, width):
            ci, sub = divmod(off, LOADCH)
            return xt[ci][:, sub:sub + width]

        # ---------------- masks (emitted early; built in bf16) -------------
        wA = singles.tile([C, C], F32)
        nc.scalar.activation(wA[:], lg[:], AF.Tanh)
        wT_ps = pt_tile()
        nc.tensor.transpose(wT_ps[:, :C], wA[:], ident[:])
        wT = singles.tile([C, C], F32)
        nc.scalar.copy(wT[:], wT_ps[:, :C])

        def build_masks(dst, src, nbins):
            # dst[:, j*128:(j+1)*128] = mask_j computed from src [128,128]
            for j in range(8):
                lo = -1.0 + 0.25 * j
                nc.vector.tensor_scalar(dst[:, j * C:(j + 1) * C], src[:],
                                        float(lo), None, ALU.is_gt)
            for j in range(7):
                nc.vector.tensor_tensor(dst[:, j * C:(j + 1) * C],
                                        dst[:, j * C:(j + 1) * C],
                                        dst[:, (j + 1) * C:(j + 2) * C],
                                        ALU.subtract)
            neq = singles.tile([C, C], BF16, tag=f"neq{nbins}",
                               name=f"neq{nbins}")
            nc.vector.tensor_scalar(neq[:], src[:], 0.0, None, ALU.not_equal)
            nc.vector.tensor_tensor(dst[:, 3 * C:4 * C], dst[:, 3 * C:4 * C],
                                    neq[:], ALU.mult)
            if nbins > 8:
                nc.vector.memset(dst[:, 8 * C:9 * C], 1.0)

        mT = singles.tile([C, 9 * C], BF16)    # [f, j*128 + p]; j=8 -> ones
        build_masks(mT, wT, 9)
        mA = singles.tile([C, 8 * C], BF16)    # A-layout: [p, j*128+f]
        build_masks(mA, wA, 8)

        # ---------------- pooled: bf16 transposes + accumulating matmuls ---
        # pooledT[j, c] (j=8 row = sum_x) accumulated over 128 chunks of n.
        ones_col = singles.tile([C, 1], F32)
        nc.vector.memset(ones_col[:], 1.0)
        # PE warm-up gated on the first x chunk: opens the HAM clock gate
        # before the transpose/pool matmul stream begins.
        for wi in range(34):
            wp = pt_tile()
            nc.tensor.matmul(wp[:, :C], xt[0][:, :C], ident_bf[:],
                             start=True, stop=True)
        pooledT_ps = psB.tile([K + 1, C], F32)
        NGR = 4  # chunks per transpose group
        for g in range(C // NGR):
            tp = pt_tile()
            for i in range(NGR):
                q = g * NGR + i
                nc.tensor.matmul(tp[:, i * C:(i + 1) * C],
                                 xsl(q * C, C), ident_bf[:],
                                 start=True, stop=True)
            xT = xtp.tile([C, NGR * C], BF16, tag="xT", name="xT")
            if g % 4 == 1:
                nc.scalar.copy(xT[:], tp[:])
            else:
                nc.vector.tensor_copy(xT[:], tp[:])
            for i in range(NGR):
                q = g * NGR + i
                nc.tensor.matmul(pooledT_ps[:], mT[:, q::C],
                                 xT[:, i * C:(i + 1) * C],
                                 start=(q == 0), stop=(q == C - 1))

        # ---------------- x sumsq (bf16 stream, scalar engine) -------------
        xsq_part = singles.tile([C, N // LOADCH], F32)
        for ci in range(N // LOADCH):
            scr = xtp.tile([C, LOADCH], BF16, tag="sqscr", name="sqscr")
            nc.scalar.activation(scr[:], xt[ci][:], AF.Square,
                                 accum_out=xsq_part[:, ci:ci + 1])

        # ---------------- Mrow (bf16) via HWDGE DRAM roundtrip -------------
        mrow_dram = dramp.tile([K, N], BF16)
        for j in range(K):
            nc.sync.dma_start(mrow_dram[j:j + 1, :].rearrange("o n -> (o n)"),
                              mA[:, j * C:(j + 1) * C])
        Mrow = singles.tile([K, N], BF16)
        nc.sync.dma_start(Mrow[:], mrow_dram[:])

        # nums: per-bin counts. numsA[p, j] = sum_f mA[p, j*128+f]
        numsA = singles.tile([C, K], F32)
        for j in range(K):
            nc.vector.reduce_sum(numsA[:, j:j + 1], mA[:, j * C:(j + 1) * C],
                                 axis=AX.X)
        nums_ps = psD.tile([K, 1], F32, tag="pd", name="nums_ps")
        nc.tensor.matmul(nums_ps[:], numsA[:], ones_col[:], start=True,
                         stop=True)
        nums_c = singles.tile([K, 1], F32)   # counts, col [j, 1]
        nc.vector.tensor_copy(nums_c[:], nums_ps[:])
        rnums_c = singles.tile([K, 1], F32)
        nc.vector.tensor_scalar(rnums_c[:], nums_c[:], 1.0, None, ALU.max)
        nc.vector.reciprocal(rnums_c[:], rnums_c[:])

        # ---------------- weights / vectors prep (off critical path) -------
        temp = float(np.sqrt(np.float32(C)))
        wsb = {}
        for nm in w_d:
            t = singles.tile([C, C], F32, tag=f"wl_{nm}", name=f"wl_{nm}")
            nc.sync.dma_start(t[:], w_d[nm].ap())
            wsb[nm] = t
        fc1w = singles.tile([C // 2, C], F32)
        nc.sync.dma_start(fc1w[:], fc1w_d.ap())
        fc2w = singles.tile([C, C // 2], F32)
        nc.sync.dma_start(fc2w[:], fc2w_d.ap())

        wqkT = []
        wvT = []
        for l in range(3):
            qk = singles.tile([C, 2 * C], BF16, tag=f"wqkT{l}",
                              name=f"wqkT{l}")
            for s, nm in enumerate((f"Wq{l+1}", f"Wk{l+1}")):
                ps = pt_tile()
                nc.tensor.transpose(ps[:, :C], wsb[nm][:], ident[:])
                nc.scalar.activation(qk[:, s * C:(s + 1) * C], ps[:, :C],
                                     AF.Copy,
                                     scale=(1.0 / temp if s == 0 else 1.0))
            wqkT.append(qk)
            vt = singles.tile([C, C], BF16, tag=f"wvT{l}", name=f"wvT{l}")
            ps = pt_tile()
            nc.tensor.transpose(ps[:, :C], wsb[f"Wv{l+1}"][:], ident[:])
            nc.scalar.copy(vt[:], ps[:, :C])
            wvT.append(vt)
        convwT = singles.tile([C, C], F32)
        ps = pt_tile()
        nc.tensor.transpose(ps[:, :C], wsb["conv0_w"][:], ident[:])
        nc.scalar.copy(convwT[:], ps[:, :C])
        convwT_bf = singles.tile([C, C], BF16)
        nc.vector.tensor_copy(convwT_bf[:], convwT[:])
        fc1wT = singles.tile([C, C // 2], F32)
        ps = pt_tile()
        nc.tensor.transpose(ps[:, :C // 2], fc1w[:], ident[:C // 2, :C // 2])
        nc.scalar.copy(fc1wT[:], ps[:, :C // 2])
        fc2wT = singles.tile([C // 2, C], F32)
        ps = pt_tile()
        nc.tensor.transpose(ps[:C // 2, :C], fc2w[:], ident[:])
        nc.scalar.copy(fc2wT[:], ps[:C // 2, :C])

        bnw_row = singles.tile([1, C], F32)
        nc.sync.dma_start(bnw_row[:], vecs["bn_w"].ap()[None, :])
        bnb_row = singles.tile([1, C], F32)
        nc.sync.dma_start(bnb_row[:], vecs["bn_b"].ap()[None, :])
        vrows = singles.tile([5, C], F32)
        nc.vector.memset(vrows[:], 0.0)
        for r, nm in enumerate(("ln_w", "ln_b", "conv0_b", "fc2_b")):
            nc.sync.dma_start(vrows[r:r + 1, :], vecs[nm].ap()[None, :])
        nc.sync.dma_start(vrows[4:5, :C // 2], vecs["fc1_b"].ap()[None, :])
        ps = pt_tile()
        nc.tensor.transpose(ps[:, :5], vrows[:], ident[:5, :5])
        vcols = singles.tile([C, 5], F32)
        nc.scalar.copy(vcols[:], ps[:, :5])
        lnw_c, lnb_c = vcols[:, 0:1], vcols[:, 1:2]
        convb_c, fc2b_c = vcols[:, 2:3], vcols[:, 3:4]
        fc1b_c = vcols[:C // 2, 4:5]
        fc2b_half = singles.tile([C, 1], F32)
        nc.vector.tensor_scalar(fc2b_half[:], fc2b_c, 0.5, None, ALU.mult)
        ones_row = singles.tile([1, C], F32)
        nc.vector.memset(ones_row[:], 1.0)
        ones8 = singles.tile([K, 1], F32)
        nc.vector.memset(ones8[:], 1.0)
        ones9 = singles.tile([K + 1, 1], F32)
        nc.vector.memset(ones9[:], 1.0)
        neg8 = singles.tile([K, 1], F32)
        nc.vector.memset(neg8[:], -1.0)

        # ---------------- pooled epilogue ----------------
        pooledT9 = singles.tile([K + 1, C], F32)
        nc.vector.tensor_copy(pooledT9[:], pooledT_ps[:])
        pooledT = pooledT9[:K, :]
        # sumx_row = (sum of all 9 rows) - (sum of the 8 bin rows)
        s9_ps = psD.tile([1, C], F32, tag="pd", name="s9_ps")
        nc.tensor.matmul(s9_ps[:], ones9[:], pooledT9[:], start=True,
                         stop=False)
        nc.tensor.matmul(s9_ps[:], neg8[:], pooledT, start=False, stop=True)
        sumx_row = singles.tile([1, C], F32)
        nc.vector.tensor_copy(sumx_row[:], s9_ps[:])

        featT = singles.tile([K, C], BF16)
        nc.vector.tensor_scalar(featT[:], pooledT, rnums_c[:], None,
                                ALU.mult)

        # ---------------- attention x3 (fea orientation [c, j], bf16) ------
        fea = singles.tile([C, K], BF16, tag="fea0", name="fea0")
        fps = pt_tile()
        nc.tensor.matmul(fps[:, :K], featT[:], ident_bf[:K, :K],
                         start=True, stop=True)
        nc.vector.tensor_copy(fea[:], fps[:, :K])

        for l in range(3):
            qk_ps = pt_tile()
            nc.tensor.matmul(qk_ps[:K, :2 * C], fea[:], wqkT[l][:],
                             start=True, stop=True)
            qkT = singles.tile([K, 2 * C], BF16, tag=f"qkT{l}",
                               name=f"qkT{l}")
            nc.vector.tensor_copy(qkT[:], qk_ps[:K, :2 * C])
            v_ps = pt_tile()
            nc.tensor.matmul(v_ps[:, :K], wvT[l][:], fea[:], start=True,
                             stop=True)
            vsb = singles.tile([C, K], BF16, tag=f"v{l}", name=f"v{l}")
            nc.vector.tensor_copy(vsb[:], v_ps[:, :K])

            at_ps = pt_tile()
            nc.tensor.matmul(at_ps[:, :C], qkT[:, :C], qkT[:, C:],
                             start=True, stop=True)
            esb = singles.tile([C, C], BF16, tag=f"e{l}", name=f"e{l}")
            sume = singles.tile([C, 1], F32, tag=f"se{l}", name=f"se{l}")
            nc.scalar.activation(esb[:], at_ps[:, :C], AF.Exp,
                                 accum_out=sume[:])
            rse = singles.tile([C, 1], F32, tag=f"rse{l}", name=f"rse{l}")
            nc.vector.reciprocal(rse[:], sume[:])
            eT_ps = pt_tile()
            nc.tensor.matmul(eT_ps[:, :C], esb[:], ident_bf[:],
                             start=True, stop=True)
            eT = singles.tile([C, C], BF16, tag=f"eT{l}", name=f"eT{l}")
            nc.scalar.copy(eT[:], eT_ps[:, :C])
            ao_ps = psD.tile([C, K], F32, tag="pd", name=f"ao_ps{l}")
            nc.tensor.matmul(ao_ps[:], eT[:], vsb[:], start=True, stop=True)

            osb = singles.tile([C, 2 * K], F32, tag=f"osb{l}",
                               name=f"osb{l}")
            nc.vector.tensor_scalar(osb[:, :K], ao_ps[:], rse[:], None,
                                    ALU.mult)
            nc.vector.tensor_tensor(osb[:, :K], osb[:, :K], fea[:], ALU.add)
            nc.vector.tensor_tensor(osb[:, K:], osb[:, :K], osb[:, :K],
                                    ALU.mult)
            st_ps = psD.tile([1, 2 * K], F32, tag="pd", name=f"st_ps{l}")
            nc.tensor.matmul(st_ps[:], ones_col[:], osb[:], start=True,
                             stop=True)
            mr = singles.tile([1, 2 * K], F32, tag=f"mr{l}", name=f"mr{l}")
            # mr = [mean | meansq+eps]
            nc.vector.tensor_scalar(mr[:, :K], st_ps[:, :K], 1.0 / C, None,
                                    ALU.mult)
            nc.vector.tensor_scalar(mr[:, K:], st_ps[:, K:], 1.0 / C, 1e-6,
                                    ALU.mult, ALU.add)
            musq = singles.tile([1, K], F32, tag=f"musq{l}", name=f"musq{l}")
            nc.vector.tensor_tensor(musq[:], mr[:, :K], mr[:, :K], ALU.mult)
            nc.vector.tensor_tensor(mr[:, K:], mr[:, K:], musq[:],
                                    ALU.subtract)
            rsqrt_inplace(mr[:, K:], K, iters=1)
            bc_ps = pt_tile()
            nc.tensor.matmul(bc_ps[:, :2 * K], ones_row[:], mr[:],
                             start=True, stop=True)
            fea2 = singles.tile([C, K], BF16, tag=f"fea{l+1}",
                                name=f"fea{l+1}")
            fsc = singles.tile([C, K], F32, tag=f"fsc{l}", name=f"fsc{l}")
            nc.vector.tensor_tensor(fsc[:], osb[:, :K], bc_ps[:, :K],
                                    ALU.subtract)
            nc.vector.tensor_tensor(fsc[:], fsc[:], bc_ps[:, K:2 * K],
                                    ALU.mult)
            nc.vector.tensor_scalar(fea2[:], fsc[:], lnw_c, lnb_c,
                                    ALU.mult, ALU.add)
            fea = fea2

        # exit transpose: featT_f [j, c] (+ squared) for scatter & stats
        ftp = pt_tile()
        nc.tensor.matmul(ftp[:K, :C], fea[:], ident_bf[:], start=True,
                         stop=True)
        featT2 = singles.tile([K, 2 * C], F32)   # [featT | featT^2]
        nc.vector.tensor_copy(featT2[:, :C], ftp[:K, :C])
        nc.vector.tensor_tensor(featT2[:, C:], featT2[:, :C], featT2[:, :C],
                                ALU.mult)
        featT2_bf = singles.tile([K, C], BF16)
        nc.vector.tensor_copy(featT2_bf[:], featT2[:, :C])

        # PE warm-up: ~3.5us of back-to-back matmuls gated on featT2_bf so
        # the HAM clock gate opens (K=8/8) right before pass R's matmuls.
        for wi in range(34):
            wp = pt_tile()
            nc.tensor.matmul(wp[:, :C], featT2_bf[:],
                             ident_bf[:K, :], start=True, stop=True)

        # ---------------- closed-form instance stats ----------------
        # packed rows: [vpe | rbn_arg] -> one Newton rsqrt over 2C
        prod = singles.tile([K, C], F32)
        nc.vector.tensor_tensor(prod[:], featT2[:, :C], pooledT,
                                ALU.mult)
        r12_ps = psD.tile([1, 2 * C], F32, tag="pd", name="r12_ps")
        nc.tensor.matmul(r12_ps[:], nums_c[:], featT2[:], start=True,
                         stop=True)
        r3_ps = pt_tile()
        nc.tensor.matmul(r3_ps[:1, :C], ones8[:], prod[:], start=True,
                         stop=True)

        xsq_col = singles.tile([C, 1], F32)
        nc.vector.reduce_sum(xsq_col[:], xsq_part[:], axis=AX.X)
        xsqr_ps = pt_tile()
        nc.tensor.transpose(xsqr_ps[:1, :C], xsq_col[:], ident[:])
        srow = singles.tile([1, C], F32)
        nc.vector.tensor_tensor(srow[:], sumx_row[:], r12_ps[:, :C],
                                ALU.add)
        ssrow = singles.tile([1, C], F32)
        nc.vector.tensor_scalar(ssrow[:], r3_ps[:1, :C], 2.0, None, ALU.mult)
        nc.vector.tensor_tensor(ssrow[:], ssrow[:], r12_ps[:, C:], ALU.add)
        nc.vector.tensor_tensor(ssrow[:], ssrow[:], xsqr_ps[:1, :C], ALU.add)
        mu_row = singles.tile([1, C], F32)
        nc.vector.tensor_scalar(mu_row[:], srow[:], 1.0 / N, None, ALU.mult)
        musq_row = singles.tile([1, C], F32)
        nc.vector.tensor_tensor(musq_row[:], mu_row[:], mu_row[:], ALU.mult)
        var_row = singles.tile([1, C], F32)
        nc.vector.tensor_scalar(var_row[:], ssrow[:], 1.0 / N, None, ALU.mult)
        nc.vector.tensor_tensor(var_row[:], var_row[:], musq_row[:],
                                ALU.subtract)
        pk = singles.tile([1, 2 * C], F32)   # [vpe | var/(var+eps)+eps]
        nc.vector.tensor_scalar(pk[:, :C], var_row[:], 1e-5, None, ALU.add)
        inv = singles.tile([1, C], F32)
        nc.vector.reciprocal(inv[:], pk[:, :C])
        nc.vector.tensor_tensor(pk[:, C:], var_row[:], inv[:], ALU.mult)
        nc.vector.tensor_scalar(pk[:, C:], pk[:, C:], 1e-5, None, ALU.add)
        rsqrt_inplace(pk[:], 2 * C)          # -> [rs_i | rs_b]
        s_rowt = singles.tile([1, C], F32)
        b_rowt = singles.tile([1, C], F32)
        nc.vector.tensor_tensor(s_rowt[:], pk[:, :C], pk[:, C:], ALU.mult)
        nc.vector.tensor_tensor(s_rowt[:], s_rowt[:], bnw_row[:], ALU.mult)
        nc.vector.tensor_tensor(b_rowt[:], mu_row[:], s_rowt[:], ALU.mult)
        nc.vector.tensor_tensor(b_rowt[:], bnb_row[:], b_rowt[:],
                                ALU.subtract)
        s_ps = psD.tile([C, 2], F32, tag="pd", name="s_ps")
        nc.tensor.transpose(s_ps[:, :1], s_rowt[:], ident[:1, :1])
        b_ps = pt_tile()
        nc.tensor.transpose(b_ps[:, :1], b_rowt[:], ident[:1, :1])
        sb_col = singles.tile([C, 2], F32)
        nc.vector.tensor_copy(sb_col[:, 0:1], s_ps[:, :1])
        nc.vector.tensor_copy(sb_col[:, 1:2], b_ps[:, :1])
        s_col, b_col = sb_col[:, 0:1], sb_col[:, 1:2]

        # ---------------- pass R: scatter matmul + DVE x-add + gelu --------
        # recon overwrites xb in place (DVE); scalar gelu reads SBUF and
        # accumulates the SE mean via accum_out.
        gsum_part = singles.tile([C, N // RCH], F32)
        gt = []
        for r in range(N // RCH):
            off = r * RCH
            sc_ps = psR.tile([C, RCH], F32, tag="pr", name="sc_ps")
            for h in range(2):
                sl = off + h * 512
                nc.tensor.matmul(sc_ps[:, h * 512:(h + 1) * 512],
                                 featT2_bf[:], Mrow[:, sl:sl + 512],
                                 start=True, stop=True)
            xr = xsl(off, RCH)
            nc.vector.tensor_tensor(xr, sc_ps[:], xr, ALU.add)
            g = gpool.tile([C, RCH], BF16, tag=f"g{r}", name=f"g{r}")
            nc.scalar.activation(g[:], xr, AF.Gelu, bias=b_col,
                                 scale=s_col, accum_out=gsum_part[:, r:r + 1])
            gt.append(g)

        # ---------------- SE gates ----------------
        gsum_col = singles.tile([C, 1], F32)
        nc.vector.reduce_sum(gsum_col[:], gsum_part[:], axis=AX.X)
        sq_ps = psD.tile([C, 1], F32, tag="pd", name="sq_ps")
        nc.tensor.matmul(sq_ps[:], convwT[:], gsum_col[:], start=True,
                         stop=True)
        sq = singles.tile([C, 1], F32)
        nc.vector.tensor_scalar(sq[:], sq_ps[:], 1.0 / N, convb_c,
                                ALU.mult, ALU.add)
        f1_ps = psD.tile([C // 2, 1], F32, tag="pd", name="f1_ps")
        nc.tensor.matmul(f1_ps[:], fc1wT[:], sq[:], start=True, stop=True)
        f1 = singles.tile([C // 2, 1], F32)
        nc.scalar.activation(f1[:], f1_ps[:], AF.Gelu, bias=fc1b_c)
        f2_ps = psD.tile([C, 1], F32, tag="pd", name="f2_ps")
        nc.tensor.matmul(f2_ps[:], fc2wT[:], f1[:], start=True, stop=True)
        # sigmoid(z) = 0.5 + 0.5*tanh(z/2) -- stays on the gelu table set
        f2 = singles.tile([C, 1], F32)
        nc.scalar.activation(f2[:], f2_ps[:], AF.Tanh, scale=0.5,
                             bias=fc2b_half[:])
        nc.vector.tensor_scalar(f2[:], f2[:], 0.5, 0.5, ALU.mult, ALU.add)
        fb = singles.tile([C, 1], F32)     # f2 * conv0_b
        nc.vector.tensor_tensor(fb[:], f2[:], convb_c, ALU.mult)

        # ---------------- pass F: conv + gate + store ----------------
        for r in range(N // RCH):
            off = r * RCH
            cv_ps = psR.tile([C, RCH], F32, tag="pr", name="cv_ps")
            for h in range(2):
                nc.tensor.matmul(cv_ps[:, h * 512:(h + 1) * 512],
                                 convwT_bf[:], gt[r][:, h * 512:(h + 1) * 512],
                                 start=True, stop=True)
            ot = och.tile([C, RCH], F32, tag="ot", name="ot")
            if r % 2 == 0:
                nc.scalar.activation(ot[:], cv_ps[:], AF.Identity, bias=fb[:],
                                     scale=f2[:])
            else:
                nc.vector.tensor_scalar(ot[:], cv_ps[:], f2[:], fb[:],
                                        ALU.mult, ALU.add)
            nc.sync.dma_start(out_d.ap()[:, off:off + RCH], ot[:])


_NC_CACHE = {}


def _get_nc():
    if "nc" not in _NC_CACHE:
        _NC_CACHE["nc"] = build_nc()
    return _NC_CACHE["nc"]


def kernel(**inputs):
    x = np.ascontiguousarray(np.asarray(inputs["x"], dtype=np.float32))
    logits = np.ascontiguousarray(np.asarray(inputs["logits"],
                                             dtype=np.float32))
    assert x.shape == (B, C, N, 1) and logits.shape == (B, N)
    ident = np.eye(C, dtype=np.float32)
    shared = {"ident": ident}
    for nm in ("Wq1", "Wk1", "Wv1", "Wq2", "Wk2", "Wv2", "Wq3", "Wk3", "Wv3",
               "conv0_w", "fc1_w", "fc2_w", "ln_w", "ln_b", "bn_w", "bn_b",
               "conv0_b", "fc1_b", "fc2_b"):
        shared[nm] = np.ascontiguousarray(np.asarray(inputs[nm],
                                                     dtype=np.float32))
    in_maps = []
    for i in range(NCORES):
        m = dict(shared)
        m["x"] = np.ascontiguousarray(x[i, :, :, 0])
        m["logits"] = np.ascontiguousarray(logits[i])
        in_maps.append(m)

    nc = _get_nc()
    res = run_bass_kernel_spmd(nc, in_maps, list(range(NCORES))).results
    out = np.stack([res[i]["out"] for i in range(NCORES)], axis=0)
    return out[..., None].astype(np.float32)


# revision 25
# speedup vs baseline: 1.8661x; 1.0689x over previous
"""Trainium2 Bass kernel for nn_GSA_74045236183284 (histogram_binning).

Sharding: data-parallel over batch B=8 across 8 NeuronCores (1 sample/core).
All params replicated. Zero collectives: BatchNorm batch-variance is
approximated by the local sample's var/(var+eps) (deviation <3e-6 rel).
InstanceNorm statistics are computed in closed form from bin sums/counts and
sum(x)/sum(x^2), avoiding extra passes over the 8MB stream.

v4 notes:
 - x loaded via SWDGE casting DMA straight to bf16 (loads issued first);
   mask pipeline (tanh -> transpose -> bin compare) emitted before weight
   prep so pooling can start as soon as the first chunks land.
 - all streaming matmuls (transpose, pool, scatter, conv) run bf16.
 - recon add (x + scatter) folded into the scatter matmul as an accumulating
   identity matmul -> PSUM; scalar gelu reads PSUM directly, 1024 wide.
 - exactly 2 activation table loads (exp_and_others / gelu_and_others);
   every rsqrt is a DVE Newton iteration, sigmoid is 0.5+0.5*tanh(x/2).
 - PSUM: pt[2]+pooled[1]+pd[1]+pr[2x2] = 8 banks.
"""

import sys

for _p in ("/opt/trn_rl_repo",):
    if _p not in sys.path:
        sys.path.insert(0, _p)

import numpy as np

import concourse.bass as bass
import concourse.bacc as bacc
import concourse.mybir as mybir
import concourse.tile as tile
from concourse.bass_utils import run_bass_kernel_spmd

F32 = mybir.dt.float32
BF16 = mybir.dt.bfloat16
I32 = mybir.dt.int32
AF = mybir.ActivationFunctionType
ALU = mybir.AluOpType
AX = mybir.AxisListType

B, C, N, K = 8, 128, 16384, 8
NCORES = 8
LOADCH = 2048   # x cast-load chunk (8 chunks)
RCH = 1024      # scatter/conv chunk (16 chunks)
MAGIC = 0x5F3759DF


def build_nc():
    nc = bacc.Bacc("TRN2", target_bir_lowering=False, debug=False,
                   num_devices=NCORES)

    x_d = nc.dram_tensor("x", [C, N], F32, kind="ExternalInput")
    logits_d = nc.dram_tensor("logits", [N], F32, kind="ExternalInput")
    ident_d = nc.dram_tensor("ident", [C, C], F32, kind="ExternalInput")
    w_d = {}
    for nm in ("Wq1", "Wk1", "Wv1", "Wq2", "Wk2", "Wv2", "Wq3", "Wk3", "Wv3",
               "conv0_w"):
        w_d[nm] = nc.dram_tensor(nm, [C, C], F32, kind="ExternalInput")
    fc1w_d = nc.dram_tensor("fc1_w", [C // 2, C], F32, kind="ExternalInput")
    fc2w_d = nc.dram_tensor("fc2_w", [C, C // 2], F32, kind="ExternalInput")
    vecs = {}
    for nm in ("ln_w", "ln_b", "bn_w", "bn_b", "conv0_b", "fc2_b"):
        vecs[nm] = nc.dram_tensor(nm, [C], F32, kind="ExternalInput")
    vecs["fc1_b"] = nc.dram_tensor("fc1_b", [C // 2], F32, kind="ExternalInput")
    out_d = nc.dram_tensor("out", [C, N], F32, kind="ExternalOutput")

    with tile.TileContext(nc) as tc:
        _body(tc, nc, x_d, logits_d, ident_d, w_d, fc1w_d, fc2w_d, vecs, out_d)

    nc.compile()
    return nc


def _body(tc, nc, x_d, logits_d, ident_d, w_d, fc1w_d, fc2w_d, vecs, out_d):
    from contextlib import ExitStack
    ctx = ExitStack()
    with ctx:
        singles = ctx.enter_context(tc.tile_pool(name="singles", bufs=1))
        xpool = ctx.enter_context(tc.tile_pool(name="xpool", bufs=1))
        gpool = ctx.enter_context(tc.tile_pool(name="gpool", bufs=1))
        xtp = ctx.enter_context(tc.tile_pool(name="xtp", bufs=3))
        och = ctx.enter_context(tc.tile_pool(name="och", bufs=3))
        dramp = ctx.enter_context(tc.tile_pool(name="dramp", bufs=1, space="DRAM"))
        psT = ctx.enter_context(tc.tile_pool(name="psT", bufs=2, space="PSUM"))
        psB = ctx.enter_context(tc.tile_pool(name="psB", bufs=1, space="PSUM"))
        psD = ctx.enter_context(tc.tile_pool(name="psD", bufs=1, space="PSUM"))
        psR = ctx.enter_context(tc.tile_pool(name="psR", bufs=2, space="PSUM"))

        def pt_tile():
            return psT.tile([C, 4 * C], F32, tag="pt", name="pt")

        # ---------------- DVE Newton rsqrt helper (no ACT table) -----------
        nw_scr = {}

        def rsqrt_inplace(row, width, iters=2):
            """row[:1,:width] := 1/sqrt(row) via int bit-trick + Newton.
            iters=1 -> ~0.2% rel err, iters=2 -> ~5e-6."""
            if width not in nw_scr:
                iv = singles.tile([1, width], I32, tag=f"nw_i{width}",
                                  name=f"nw_i{width}")
                mg = singles.tile([1, width], I32, tag=f"nw_m{width}",
                                  name=f"nw_m{width}")
                nc.vector.memset(mg[:], MAGIC)
                t = singles.tile([1, width], F32, tag=f"nw_t{width}",
                                 name=f"nw_t{width}")
                nw_scr[width] = (iv, mg, t)
            iv, mg, t = nw_scr[width]
            nc.vector.tensor_scalar(iv[:], row.bitcast(I32), 1, None,
                                    ALU.logical_shift_right)
            nc.vector.tensor_tensor(iv[:], mg[:], iv[:], ALU.subtract)
            y = iv[:].bitcast(F32)
            for it in range(iters):
                nc.vector.tensor_tensor(t[:], y, y, ALU.mult)
                nc.vector.tensor_tensor(t[:], t[:], row, ALU.mult)
                nc.vector.tensor_scalar(t[:], t[:], -0.5, 1.5,
                                        ALU.mult, ALU.add)
                if it == iters - 1:
                    nc.vector.tensor_tensor(row, y, t[:], ALU.mult)
                else:
                    nc.vector.tensor_tensor(iv[:].bitcast(F32), y, t[:],
                                            ALU.mult)

        # ---------------- loads first: logits, identity, x ----------------
        lg = singles.tile([C, C], F32)   # logits as [p, f], n = p*128+f
        nc.sync.dma_start(lg[:], logits_d.ap().rearrange("(p f) -> p f", f=C))
        ident = singles.tile([C, C], F32)
        nc.sync.dma_start(ident[:], ident_d.ap())
        ident_bf = singles.tile([C, C], BF16)
        nc.vector.tensor_copy(ident_bf[:], ident[:])

        xt = []
        for ci in range(N // LOADCH):
            t = xpool.tile([C, LOADCH], BF16, tag=f"x{ci}", name=f"x{ci}")
            nc.gpsimd.dma_start(t[:], x_d.ap()[:, ci * LOADCH:(ci + 1) * LOADCH])
            xt.append(t)

        def xsl(off, width):
            ci, sub = divmod(off, LOADCH)
            return xt[ci][:, sub:sub + width]

        # ---------------- masks (emitted early; built in bf16) -------------
        wA = singles.tile([C, C], F32)
        nc.scalar.activation(wA[:], lg[:], AF.Tanh)
        wT_ps = pt_tile()
        nc.tensor.transpose(wT_ps[:, :C], wA[:], ident[:])
        wT = singles.tile([C, C], F32)
        nc.scalar.copy(wT[:], wT_ps[:, :C])

        def build_masks(dst, src, nbins):
            # dst[:, j*128:(j+1)*128] = mask_j computed from src [128,128]
            for j in range(8):
                lo = -1.0 + 0.25 * j
                nc.vector.tensor_scalar(dst[:, j * C:(j + 1) * C], src[:],
                                        float(lo), None, ALU.is_gt)
            for j in range(7):
                nc.vector.tensor_tensor(dst[:, j * C:(j + 1) * C],
                                        dst[:, j * C:(j + 1) * C],
                                        dst[:, (j + 1) * C:(j + 2) * C],
                                        ALU.subtract)
            neq = singles.tile([C, C], BF16, tag=f"neq{nbins}",
                               name=f"neq{nbins}")
            nc.vector.tensor_scalar(neq[:], src[:], 0.0, None, ALU.not_equal)
            nc.vector.tensor_tensor(dst[:, 3 * C:4 * C], dst[:, 3 * C:4 * C],
                                    neq[:], ALU.mult)
            if nbins > 8:
                nc.vector.memset(dst[:, 8 * C:9 * C], 1.0)

        mT = singles.tile([C, 9 * C], BF16)    # [f, j*128 + p]; j=8 -> ones
        build_masks(mT, wT, 9)
        mA = singles.tile([C, 8 * C], BF16)    # A-layout: [p, j*128+f]
        build_masks(mA, wA, 8)

        # ---------------- pooled: bf16 transposes + accumulating matmuls ---
        # pooledT[j, c] (j=8 row = sum_x) accumulated over 128 chunks of n.
        ones_col = singles.tile([C, 1], F32)
        nc.vector.memset(ones_col[:], 1.0)
        # Pool matmuls are packed 4-wide into the PE array via column-group
        # tile_position: chunk q accumulates into output partition group
        # 32*(q%4), so 4 pool matmuls run concurrently in the array.
        pooled4_ps = psB.tile([3 * 32 + K + 1, C], F32)
        NGR = 4  # chunks per transpose group
        for g in range(C // NGR):
            tp = pt_tile()
            for i in range(NGR):
                q = g * NGR + i
                nc.tensor.matmul(tp[:, i * C:(i + 1) * C],
                                 xsl(q * C, C), ident_bf[:],
                                 start=True, stop=True)
            xT = xtp.tile([C, NGR * C], BF16, tag="xT", name="xT")
            if g % 4 == 1:
                nc.scalar.copy(xT[:], tp[:])
            else:
                nc.vector.tensor_copy(xT[:], tp[:])
            for i in range(NGR):
                q = g * NGR + i
                grp = q % 4
                nc.tensor.matmul(pooled4_ps[32 * grp:32 * grp + K + 1, :],
                                 mT[:, q::C],
                                 xT[:, i * C:(i + 1) * C],
                                 start=(q == grp), stop=(q == C - 4 + grp),
                                 tile_position=(0, 32 * grp))

        # ---------------- x sumsq (bf16 stream, scalar engine) -------------
        xsq_part = singles.tile([C, N // LOADCH], F32)
        for ci in range(N // LOADCH):
            scr = xtp.tile([C, LOADCH], BF16, tag="sqscr", name="sqscr")
            nc.scalar.activation(scr[:], xt[ci][:], AF.Square,
                                 accum_out=xsq_part[:, ci:ci + 1])

        # ---------------- Mrow (bf16) via HWDGE DRAM roundtrip -------------
        mrow_dram = dramp.tile([K, N], BF16)
        for j in range(K):
            nc.sync.dma_start(mrow_dram[j:j + 1, :].rearrange("o n -> (o n)"),
                              mA[:, j * C:(j + 1) * C])
        Mrow = singles.tile([K, N], BF16)
        nc.sync.dma_start(Mrow[:], mrow_dram[:])

        # nums: per-bin counts. numsA[p, j] = sum_f mA[p, j*128+f]
        numsA = singles.tile([C, K], F32)
        for j in range(K):
            nc.vector.reduce_sum(numsA[:, j:j + 1], mA[:, j * C:(j + 1) * C],
                                 axis=AX.X)
        nums_ps = psD.tile([K, 1], F32, tag="pd", name="nums_ps")
        nc.tensor.matmul(nums_ps[:], numsA[:], ones_col[:], start=True,
                         stop=True)
        nums_c = singles.tile([K, 1], F32)   # counts, col [j, 1]
        nc.vector.tensor_copy(nums_c[:], nums_ps[:])
        rnums_c = singles.tile([K, 1], F32)
        nc.vector.tensor_scalar(rnums_c[:], nums_c[:], 1.0, None, ALU.max)
        nc.vector.reciprocal(rnums_c[:], rnums_c[:])

        # ---------------- weights / vectors prep (off critical path) -------
        temp = float(np.sqrt(np.float32(C)))
        wsb = {}
        for nm in w_d:
            t = singles.tile([C, C], F32, tag=f"wl_{nm}", name=f"wl_{nm}")
            nc.sync.dma_start(t[:], w_d[nm].ap())
            wsb[nm] = t
        fc1w = singles.tile([C // 2, C], F32)
        nc.sync.dma_start(fc1w[:], fc1w_d.ap())
        fc2w = singles.tile([C, C // 2], F32)
        nc.sync.dma_start(fc2w[:], fc2w_d.ap())

        wqkT = []
        wvT = []
        for l in range(3):
            qk = singles.tile([C, 2 * C], BF16, tag=f"wqkT{l}",
                              name=f"wqkT{l}")
            for s, nm in enumerate((f"Wq{l+1}", f"Wk{l+1}")):
                ps = pt_tile()
                nc.tensor.transpose(ps[:, :C], wsb[nm][:], ident[:])
                nc.scalar.activation(qk[:, s * C:(s + 1) * C], ps[:, :C],
                                     AF.Copy,
                                     scale=(1.0 / temp if s == 0 else 1.0))
            wqkT.append(qk)
            vt = singles.tile([C, C], BF16, tag=f"wvT{l}", name=f"wvT{l}")
            ps = pt_tile()
            nc.tensor.transpose(ps[:, :C], wsb[f"Wv{l+1}"][:], ident[:])
            nc.scalar.copy(vt[:], ps[:, :C])
            wvT.append(vt)
        convwT = singles.tile([C, C], F32)
        ps = pt_tile()
        nc.tensor.transpose(ps[:, :C], wsb["conv0_w"][:], ident[:])
        nc.scalar.copy(convwT[:], ps[:, :C])
        convwT_bf = singles.tile([C, C], BF16)
        nc.vector.tensor_copy(convwT_bf[:], convwT[:])
        fc1wT = singles.tile([C, C // 2], F32)
        ps = pt_tile()
        nc.tensor.transpose(ps[:, :C // 2], fc1w[:], ident[:C // 2, :C // 2])
        nc.scalar.copy(fc1wT[:], ps[:, :C // 2])
        fc2wT = singles.tile([C // 2, C], F32)
        ps = pt_tile()
        nc.tensor.transpose(ps[:C // 2, :C], fc2w[:], ident[:])
        nc.scalar.copy(fc2wT[:], ps[:C // 2, :C])

        bnw_row = singles.tile([1, C], F32)
        nc.sync.dma_start(bnw_row[:], vecs["bn_w"].ap()[None, :])
        bnb_row = singles.tile([1, C], F32)
        nc.sync.dma_start(bnb_row[:], vecs["bn_b"].ap()[None, :])
        vrows = singles.tile([5, C], F32)
        nc.vector.memset(vrows[:], 0.0)
        for r, nm in enumerate(("ln_w", "ln_b", "conv0_b", "fc2_b")):
            nc.sync.dma_start(vrows[r:r + 1, :], vecs[nm].ap()[None, :])
        nc.sync.dma_start(vrows[4:5, :C // 2], vecs["fc1_b"].ap()[None, :])
        ps = pt_tile()
        nc.tensor.transpose(ps[:, :5], vrows[:], ident[:5, :5])
        vcols = singles.tile([C, 5], F32)
        nc.scalar.copy(vcols[:], ps[:, :5])
        lnw_c, lnb_c = vcols[:, 0:1], vcols[:, 1:2]
        convb_c, fc2b_c = vcols[:, 2:3], vcols[:, 3:4]
        fc1b_c = vcols[:C // 2, 4:5]
        fc2b_half = singles.tile([C, 1], F32)
        nc.vector.tensor_scalar(fc2b_half[:], fc2b_c, 0.5, None, ALU.mult)
        ones_row = singles.tile([1, C], F32)
        nc.vector.memset(ones_row[:], 1.0)
        ones8 = singles.tile([K, 1], F32)
        nc.vector.memset(ones8[:], 1.0)
        ones9 = singles.tile([K + 1, 1], F32)
        nc.vector.memset(ones9[:], 1.0)
        neg8 = singles.tile([K, 1], F32)
        nc.vector.memset(neg8[:], -1.0)

        # ---------------- pooled epilogue: combine the 4 column groups -----
        comb = singles.tile([3 * 32 + K + 1, K + 1], F32)
        for grp in range(4):
            nc.sync.dma_start(comb[32 * grp:32 * grp + K + 1, :],
                              ident_d.ap()[:K + 1, :K + 1])
        pooled4_sb = singles.tile([3 * 32 + K + 1, C], F32)
        for grp in range(4):
            sl = slice(32 * grp, 32 * grp + K + 1)
            nc.vector.tensor_copy(pooled4_sb[sl, :], pooled4_ps[sl, :])
        comb_ps = psD.tile([K + 1, C], F32, tag="pd", name="comb_ps")
        for grp in range(4):
            sl = slice(32 * grp, 32 * grp + K + 1)
            nc.tensor.matmul(comb_ps[:], comb[sl, :], pooled4_sb[sl, :],
                             start=(grp == 0), stop=(grp == 3),
                             tile_position=(32 * grp, 0))
        pooledT9 = singles.tile([K + 1, C], F32)
        nc.vector.tensor_copy(pooledT9[:], comb_ps[:])
        pooledT = pooledT9[:K, :]
        # sumx_row = (sum of all 9 rows) - (sum of the 8 bin rows)
        s9_ps = psD.tile([1, C], F32, tag="pd", name="s9_ps")
        nc.tensor.matmul(s9_ps[:], ones9[:], pooledT9[:], start=True,
                         stop=False)
        nc.tensor.matmul(s9_ps[:], neg8[:], pooledT, start=False, stop=True)
        sumx_row = singles.tile([1, C], F32)
        nc.vector.tensor_copy(sumx_row[:], s9_ps[:])

        featT = singles.tile([K, C], BF16)
        nc.vector.tensor_scalar(featT[:], pooledT, rnums_c[:], None,
                                ALU.mult)

        # ---------------- attention x3 (fea orientation [c, j], bf16) ------
        # high_priority: the serial attention/stats chain must win engine
        # queues over the bulk pass-R stream the scheduler wants to hoist.
        _hp = tc.high_priority()
        _hp.__enter__()
        fea = singles.tile([C, K], BF16, tag="fea0", name="fea0")
        fps = pt_tile()
        nc.tensor.matmul(fps[:, :K], featT[:], ident_bf[:K, :K],
                         start=True, stop=True)
        nc.vector.tensor_copy(fea[:], fps[:, :K])

        for l in range(3):
            qk_ps = pt_tile()
            nc.tensor.matmul(qk_ps[:K, :2 * C], fea[:], wqkT[l][:],
                             start=True, stop=True)
            qkT = singles.tile([K, 2 * C], BF16, tag=f"qkT{l}",
                               name=f"qkT{l}")
            nc.vector.tensor_copy(qkT[:], qk_ps[:K, :2 * C])
            v_ps = pt_tile()
            nc.tensor.matmul(v_ps[:, :K], wvT[l][:], fea[:], start=True,
                             stop=True)
            vsb = singles.tile([C, K], BF16, tag=f"v{l}", name=f"v{l}")
            nc.vector.tensor_copy(vsb[:], v_ps[:, :K])

            at_ps = pt_tile()
            nc.tensor.matmul(at_ps[:, :C], qkT[:, :C], qkT[:, C:],
                             start=True, stop=True)
            esb = singles.tile([C, C], BF16, tag=f"e{l}", name=f"e{l}")
            sume = singles.tile([C, 1], F32, tag=f"se{l}", name=f"se{l}")
            nc.scalar.activation(esb[:], at_ps[:, :C], AF.Exp,
                                 accum_out=sume[:])
            rse = singles.tile([C, 1], F32, tag=f"rse{l}", name=f"rse{l}")
            nc.vector.reciprocal(rse[:], sume[:])
            eT_ps = pt_tile()
            nc.tensor.matmul(eT_ps[:, :C], esb[:], ident_bf[:],
                             start=True, stop=True)
            eT = singles.tile([C, C], BF16, tag=f"eT{l}", name=f"eT{l}")
            nc.scalar.copy(eT[:], eT_ps[:, :C])
            ao_ps = psD.tile([C, K], F32, tag="pd", name=f"ao_ps{l}")
            nc.tensor.matmul(ao_ps[:], eT[:], vsb[:], start=True, stop=True)

            osb = singles.tile([C, 2 * K], F32, tag=f"osb{l}",
                               name=f"osb{l}")
            nc.vector.tensor_scalar(osb[:, :K], ao_ps[:], rse[:], None,
                                    ALU.mult)
            nc.vector.tensor_tensor(osb[:, :K], osb[:, :K], fea[:], ALU.add)
            nc.vector.tensor_tensor(osb[:, K:], osb[:, :K], osb[:, :K],
                                    ALU.mult)
            st_ps = psD.tile([1, 2 * K], F32, tag="pd", name=f"st_ps{l}")
            nc.tensor.matmul(st_ps[:], ones_col[:], osb[:], start=True,
                             stop=True)
            mr = singles.tile([1, 2 * K], F32, tag=f"mr{l}", name=f"mr{l}")
            # mr = [mean | meansq+eps]
            nc.vector.tensor_scalar(mr[:, :K], st_ps[:, :K], 1.0 / C, None,
                                    ALU.mult)
            nc.vector.tensor_scalar(mr[:, K:], st_ps[:, K:], 1.0 / C, 1e-6,
                                    ALU.mult, ALU.add)
            musq = singles.tile([1, K], F32, tag=f"musq{l}", name=f"musq{l}")
            nc.vector.tensor_tensor(musq[:], mr[:, :K], mr[:, :K], ALU.mult)
            nc.vector.tensor_tensor(mr[:, K:], mr[:, K:], musq[:],
                                    ALU.subtract)
            rsqrt_inplace(mr[:, K:], K, iters=1)
            bc_ps = pt_tile()
            nc.tensor.matmul(bc_ps[:, :2 * K], ones_row[:], mr[:],
                             start=True, stop=True)
            fea2 = singles.tile([C, K], BF16, tag=f"fea{l+1}",
                                name=f"fea{l+1}")
            fsc = singles.tile([C, K], F32, tag=f"fsc{l}", name=f"fsc{l}")
            nc.vector.tensor_tensor(fsc[:], osb[:, :K], bc_ps[:, :K],
                                    ALU.subtract)
            nc.vector.tensor_tensor(fsc[:], fsc[:], bc_ps[:, K:2 * K],
                                    ALU.mult)
            nc.vector.tensor_scalar(fea2[:], fsc[:], lnw_c, lnb_c,
                                    ALU.mult, ALU.add)
            fea = fea2

        # exit transpose: featT_f [j, c] (+ squared) for scatter & stats
        ftp = pt_tile()
        nc.tensor.matmul(ftp[:K, :C], fea[:], ident_bf[:], start=True,
                         stop=True)
        featT2 = singles.tile([K, 2 * C], F32)   # [featT | featT^2]
        nc.vector.tensor_copy(featT2[:, :C], ftp[:K, :C])
        nc.vector.tensor_tensor(featT2[:, C:], featT2[:, :C], featT2[:, :C],
                                ALU.mult)
        featT2_bf = singles.tile([K, C], BF16)
        nc.vector.tensor_copy(featT2_bf[:], featT2[:, :C])

        # ---------------- closed-form instance stats ----------------
        # packed rows: [vpe | rbn_arg] -> one Newton rsqrt over 2C
        prod = singles.tile([K, C], F32)
        nc.vector.tensor_tensor(prod[:], featT2[:, :C], pooledT,
                                ALU.mult)
        r12_ps = psD.tile([1, 2 * C], F32, tag="pd", name="r12_ps")
        nc.tensor.matmul(r12_ps[:], nums_c[:], featT2[:], start=True,
                         stop=True)
        r3_ps = pt_tile()
        nc.tensor.matmul(r3_ps[:1, :C], ones8[:], prod[:], start=True,
                         stop=True)

        xsq_col = singles.tile([C, 1], F32)
        nc.vector.reduce_sum(xsq_col[:], xsq_part[:], axis=AX.X)
        xsqr_ps = pt_tile()
        nc.tensor.transpose(xsqr_ps[:1, :C], xsq_col[:], ident[:])
        srow = singles.tile([1, C], F32)
        nc.vector.tensor_tensor(srow[:], sumx_row[:], r12_ps[:, :C],
                                ALU.add)
        ssrow = singles.tile([1, C], F32)
        nc.vector.tensor_scalar(ssrow[:], r3_ps[:1, :C], 2.0, None, ALU.mult)
        nc.vector.tensor_tensor(ssrow[:], ssrow[:], r12_ps[:, C:], ALU.add)
        nc.vector.tensor_tensor(ssrow[:], ssrow[:], xsqr_ps[:1, :C], ALU.add)
        mu_row = singles.tile([1, C], F32)
        nc.vector.tensor_scalar(mu_row[:], srow[:], 1.0 / N, None, ALU.mult)
        musq_row = singles.tile([1, C], F32)
        nc.vector.tensor_tensor(musq_row[:], mu_row[:], mu_row[:], ALU.mult)
        var_row = singles.tile([1, C], F32)
        nc.vector.tensor_scalar(var_row[:], ssrow[:], 1.0 / N, None, ALU.mult)
        nc.vector.tensor_tensor(var_row[:], var_row[:], musq_row[:],
                                ALU.subtract)
        pk = singles.tile([1, 2 * C], F32)   # [vpe | var/(var+eps)+eps]
        nc.vector.tensor_scalar(pk[:, :C], var_row[:], 1e-5, None, ALU.add)
        inv = singles.tile([1, C], F32)
        nc.vector.reciprocal(inv[:], pk[:, :C])
        nc.vector.tensor_tensor(pk[:, C:], var_row[:], inv[:], ALU.mult)
        nc.vector.tensor_scalar(pk[:, C:], pk[:, C:], 1e-5, None, ALU.add)
        rsqrt_inplace(pk[:], 2 * C)          # -> [rs_i | rs_b]
        s_rowt = singles.tile([1, C], F32)
        b_rowt = singles.tile([1, C], F32)
        nc.vector.tensor_tensor(s_rowt[:], pk[:, :C], pk[:, C:], ALU.mult)
        nc.vector.tensor_tensor(s_rowt[:], s_rowt[:], bnw_row[:], ALU.mult)
        nc.vector.tensor_tensor(b_rowt[:], mu_row[:], s_rowt[:], ALU.mult)
        nc.vector.tensor_tensor(b_rowt[:], bnb_row[:], b_rowt[:],
                                ALU.subtract)
        s_ps = psD.tile([C, 2], F32, tag="pd", name="s_ps")
        nc.tensor.transpose(s_ps[:, :1], s_rowt[:], ident[:1, :1])
        b_ps = pt_tile()
        nc.tensor.transpose(b_ps[:, :1], b_rowt[:], ident[:1, :1])
        sb_col = singles.tile([C, 2], F32)
        nc.vector.tensor_copy(sb_col[:, 0:1], s_ps[:, :1])
        nc.vector.tensor_copy(sb_col[:, 1:2], b_ps[:, :1])
        s_col, b_col = sb_col[:, 0:1], sb_col[:, 1:2]
        _hp.__exit__(None, None, None)

        # ---------------- pass R: scatter matmul + DVE x-add + gelu --------
        # recon overwrites xb in place (DVE); scalar gelu reads SBUF 2048
        # wide and accumulates the SE mean via accum_out.
        NBC = N // LOADCH   # big chunks of 2048
        gsum_part = singles.tile([C, NBC], F32)
        gt = []
        for r in range(NBC):
            off = r * LOADCH
            for h in range(2):
                sc_ps = psR.tile([C, RCH], F32, tag="pr", name="sc_ps")
                for hh in range(2):
                    sl = off + h * RCH + hh * 512
                    nc.tensor.matmul(sc_ps[:, hh * 512:(hh + 1) * 512],
                                     featT2_bf[:], Mrow[:, sl:sl + 512],
                                     start=True, stop=True)
                xr = xsl(off + h * RCH, RCH)
                nc.vector.tensor_tensor(xr, sc_ps[:], xr, ALU.add)
            g = gpool.tile([C, LOADCH], BF16, tag=f"g{r}", name=f"g{r}")
            nc.scalar.activation(g[:], xt[r][:], AF.Gelu, bias=b_col,
                                 scale=s_col, accum_out=gsum_part[:, r:r + 1])
            gt.append(g)

        # ---------------- SE gates ----------------
        gsum_col = singles.tile([C, 1], F32)
        nc.vector.reduce_sum(gsum_col[:], gsum_part[:], axis=AX.X)
        sq_ps = psD.tile([C, 1], F32, tag="pd", name="sq_ps")
        nc.tensor.matmul(sq_ps[:], convwT[:], gsum_col[:], start=True,
                         stop=True)
        sq = singles.tile([C, 1], F32)
        nc.vector.tensor_scalar(sq[:], sq_ps[:], 1.0 / N, convb_c,
                                ALU.mult, ALU.add)
        f1_ps = psD.tile([C // 2, 1], F32, tag="pd", name="f1_ps")
        nc.tensor.matmul(f1_ps[:], fc1wT[:], sq[:], start=True, stop=True)
        f1 = singles.tile([C // 2, 1], F32)
        nc.scalar.activation(f1[:], f1_ps[:], AF.Gelu, bias=fc1b_c)
        f2_ps = psD.tile([C, 1], F32, tag="pd", name="f2_ps")
        nc.tensor.matmul(f2_ps[:], fc2wT[:], f1[:], start=True, stop=True)
        # sigmoid(z) = 0.5 + 0.5*tanh(z/2) -- stays on the gelu table set
        f2 = singles.tile([C, 1], F32)
        nc.scalar.activation(f2[:], f2_ps[:], AF.Tanh, scale=0.5,
                             bias=fc2b_half[:])
        nc.vector.tensor_scalar(f2[:], f2[:], 0.5, 0.5, ALU.mult, ALU.add)
        fb = singles.tile([C, 1], F32)     # f2 * conv0_b
        nc.vector.tensor_tensor(fb[:], f2[:], convb_c, ALU.mult)

        # ---------------- pass F: conv + gate + store (2048 wide) ----------
        for r in range(NBC):
            off = r * LOADCH
            ot = och.tile([C, LOADCH], F32, tag="ot", name="ot")
            for h in range(2):
                cv_ps = psR.tile([C, RCH], F32, tag="pr", name="cv_ps")
                for hh in range(2):
                    sl = h * RCH + hh * 512
                    nc.tensor.matmul(cv_ps[:, hh * 512:(hh + 1) * 512],
                                     convwT_bf[:], gt[r][:, sl:sl + 512],
                                     start=True, stop=True)
                if h == 0:
                    nc.scalar.activation(ot[:, :RCH], cv_ps[:], AF.Identity,
                                         bias=fb[:], scale=f2[:])
                else:
                    nc.vector.tensor_scalar(ot[:, RCH:], cv_ps[:], f2[:],
                                            fb[:], ALU.mult, ALU.add)
            nc.sync.dma_start(out_d.ap()[:, off:off + LOADCH], ot[:])


_NC_CACHE = {}


def _get_nc():
    if "nc" not in _NC_CACHE:
        _NC_CACHE["nc"] = build_nc()
    return _NC_CACHE["nc"]


def kernel(**inputs):
    x = np.ascontiguousarray(np.asarray(inputs["x"], dtype=np.float32))
    logits = np.ascontiguousarray(np.asarray(inputs["logits"],
                                             dtype=np.float32))
    assert x.shape == (B, C, N, 1) and logits.shape == (B, N)
    ident = np.eye(C, dtype=np.float32)
    shared = {"ident": ident}
    for nm in ("Wq1", "Wk1", "Wv1", "Wq2", "Wk2", "Wv2", "Wq3", "Wk3", "Wv3",
               "conv0_w", "fc1_w", "fc2_w", "ln_w", "ln_b", "bn_w", "bn_b",
               "conv0_b", "fc1_b", "fc2_b"):
        shared[nm] = np.ascontiguousarray(np.asarray(inputs[nm],
                                                     dtype=np.float32))
    in_maps = []
    for i in range(NCORES):
        m = dict(shared)
        m["x"] = np.ascontiguousarray(x[i, :, :, 0])
        m["logits"] = np.ascontiguousarray(logits[i])
        in_maps.append(m)

    nc = _get_nc()
    res = run_bass_kernel_spmd(nc, in_maps, list(range(NCORES))).results
    out = np.stack([res[i]["out"] for i in range(NCORES)], axis=0)
    return out[..., None].astype(np.float32)
